# revision 37
# baseline (speedup 1.0000x reference)
"""Causal self-attention (GQA + RMSNorm + RoPE) Trainium2 Bass kernel.

Sharding: data-parallel over (batch, q-rows). 8 cores = 4 batches x 2 row
sets. Each core computes full K/V for its batch and 1024 q rows chosen as
8 x 128-row tiles: core half 0 takes even tiles, half 1 odd tiles. Tiles
are processed in descending causal-extent order so that a single
compile-time key-extent schedule E = (16,14,12,10,8,6,4,2) (in 128-key
tiles) is an upper bound for both halves: total scored coverage is 72
units/head vs 68 ideal causal, vs 96 dense-halves. No collectives.

All DRAM operands are pre-arranged on the host into [128-partition,
chunk, free] layouts so every load is one contiguous run per partition
(128 descriptors instead of thousands). K-phase weights prefetch during
phase Q; proj weights prefetch during attention.

On-chip layout is channel-major: scores are computed key-major
(S^T tile = K_tile^T.T @ Q^T) in [128,8,128] PSUM strips, exp'd in one
wide ACT instruction per strip, causal-masked in place by a {0,1}
multiply on only the last two key tiles (diagonal triangle + optional
padding), and consumed by per-q-tile accumulating ys (V^T @ P) matmul
chains. The softmax denominator does one DVE fold level per strip
(w -> w/2 pairwise adds, bf16 2x mode) so the PE only streams e/2
1^T-matmuls per step instead of e. Normalization (reciprocal_approx +
gpsimd partition broadcast) is applied to the y tile.

The QKV rms/rope epilogue copies each PSUM tile to SBUF with one ACT op
immediately after the matmul chain stops, so the PE's PSUM buffer is
released after two quick ACT reads instead of being held through the
whole serial sqrt/recip/broadcast/rope chain (which previously stalled
the PE ~4us per head). The whole attention pipeline is software-
pipelined one (head, q-tile) step deep. Q stays SBUF-resident between
phases. All matmul operands bf16 (fp32 accumulate); softmax/statistics
math fp32.
"""

import os
import sys

sys.path.insert(0, "/opt/trn_rl_repo")

import ml_dtypes
import numpy as np

DIM = 2048
H = 16
HKV = 4
HD = 128
REP = H // HKV
B = 4
T = 2048
R = 1024          # q rows per core
DT = DIM // 128   # 16 contraction tiles
NJT = T // 128    # 16 key tiles
KVD = HKV * HD    # 512
EPROC = (16, 14, 12, 10, 8, 6, 4, 2)  # key-tile extent per q-tile slot
ROPE_BASE = 10000.0
EPS = float(np.finfo(np.float32).eps)
BF16 = ml_dtypes.bfloat16

_CACHE = {}


def _strips(e):
    """Split an extent into PSUM-strip chunk widths (max 8 key tiles)."""
    out = [8] * (e // 8)
    if e % 8:
        out.append(e % 8)
    return out


def _build():
    """Build + compile the SPMD Bass program (once per process)."""
    from concourse import bacc
    import concourse.mybir as mybir
    import concourse.tile as tile

    F32 = mybir.dt.float32
    BF = mybir.dt.bfloat16
    AF = mybir.ActivationFunctionType

    nc = bacc.Bacc("TRN2", target_bir_lowering=False, debug=False)

    # All tensors pre-arranged host-side: partition dim first, contiguous
    # free bytes per partition for every dma slice taken below.
    xq = nc.dram_tensor("xq", [128, DT, R], BF, kind="ExternalInput")
    xt = nc.dram_tensor("xt", [128, 4, DT, 512], BF, kind="ExternalInput")
    wq = nc.dram_tensor("wq", [128, 8, DT, 256], BF, kind="ExternalInput")
    wk = nc.dram_tensor("wk", [128, DT, KVD], BF, kind="ExternalInput")
    wv = nc.dram_tensor("wv", [128, DT, KVD], BF, kind="ExternalInput")
    wp = nc.dram_tensor("wp", [128, 4, DT, 512], BF, kind="ExternalInput")
    qgain = nc.dram_tensor("qgain", [H], F32, kind="ExternalInput")
    cosq = nc.dram_tensor("cosq", [HD, R], F32, kind="ExternalInput")
    sinq = nc.dram_tensor("sinq", [HD, R], F32, kind="ExternalInput")
    cosk = nc.dram_tensor("cosk", [HD, T], F32, kind="ExternalInput")
    sink = nc.dram_tensor("sink", [HD, T], F32, kind="ExternalInput")
    # per-core {0,1} mask for the last two key tiles of every q-tile strip:
    # half0 -> [tri, 0], half1 -> [1, tri]  (key-major [key, 2, row])
    mq = nc.dram_tensor("mq", [128, 2 * 128], BF, kind="ExternalInput")
    outT = nc.dram_tensor("outT", [DIM, R], F32, kind="ExternalOutput")

    with tile.TileContext(nc) as tc:
        with tc.tile_pool(name="const", bufs=1) as constp, \
             tc.tile_pool(name="res", bufs=1) as resp:
            # Pool stack (LIFO release): kvw, ck, rms span Q+KV; xq/wq/cq
            # are Q-only and sit on top so they can be released after Q.
            kvw_ctx = tc.tile_pool(name="kvw", bufs=1)
            kvwp = kvw_ctx.__enter__()
            ck_ctx = tc.tile_pool(name="ckp", bufs=1)
            ckp = ck_ctx.__enter__()
            rms_ctx = tc.tile_pool(name="rms", bufs=2)
            tmpp = rms_ctx.__enter__()
            ps_ctx = tc.tile_pool(name="psqkv", bufs=4, space="PSUM")
            psp = ps_ctx.__enter__()
            ss_ctx = tc.tile_pool(name="ssqkv", bufs=4, space="PSUM")
            ssp = ss_ctx.__enter__()
            xq_ctx = tc.tile_pool(name="xq", bufs=1)
            xqp = xq_ctx.__enter__()
            wq_ctx = tc.tile_pool(name="wq", bufs=2)
            wqp = wq_ctx.__enter__()
            cq_ctx = tc.tile_pool(name="cq", bufs=1)
            cqp = cq_ctx.__enter__()

            # ---- Phase-Q critical loads first (in Sync program order) ----
            # Leading chunks are small so the first matmul's deps land fast.
            xqt = xqp.tile([128, DT, R], BF)
            wq0 = wqp.tile([128, DT, 256], BF, tag="wq")
            for lo, hi in ((0, 2), (2, 4)):
                nc.sync.dma_start(out=xqt[:, lo:hi, :], in_=xq[:, lo:hi, :])
                nc.sync.dma_start(out=wq0[:, lo:hi, :], in_=wq[:, 0, lo:hi, :])
            nc.sync.dma_start(out=wq0[:, 4:DT, :], in_=wq[:, 0, 4:DT, :])
            cq = cqp.tile([128, 2, R], F32)
            nc.sync.dma_start(out=cq[:, 0], in_=cosq[:, :])
            nc.sync.dma_start(out=cq[:, 1], in_=sinq[:, :])
            for ch in range(1, 4):
                nc.sync.dma_start(out=xqt[:, ch * 4:(ch + 1) * 4, :],
                                  in_=xq[:, ch * 4:(ch + 1) * 4, :])

            # K/V-phase weights prefetch: tiles allocated here, dma_starts
            # interleaved into the phase-Q loop so queue order stays behind
            # the urgent per-group wq loads.
            wkt = kvwp.tile([128, DT, KVD], BF)
            wvt = kvwp.tile([128, DT, KVD], BF)
            ck = ckp.tile([128, 2, T], F32)

            def prefetch_kv(hg):
                if hg == 4:
                    nc.sync.dma_start(out=wkt, in_=wk[:, :, :])
                elif hg == 5:
                    nc.sync.dma_start(out=wvt, in_=wv[:, :, :])
                elif hg == 6:
                    nc.sync.dma_start(out=ck[:, 0], in_=cosk[:, :])
                    nc.sync.dma_start(out=ck[:, 1], in_=sink[:, :])

            # ---- Constants ----
            ones = constp.tile([128, 1], BF)
            nc.vector.memset(ones, 1.0)
            qg = constp.tile([1, H], F32)
            nc.sync.dma_start(out=qg, in_=qgain[None, :])
            epsq = constp.tile([1, 1], F32)
            nc.vector.memset(epsq, EPS * HD)   # q scale: 1/sqrt(ssum + HD*eps)
            epsk = constp.tile([1, 1], F32)
            nc.vector.memset(epsk, EPS)        # k scale: rsqrt(ssum/HD + eps)
            mqs = constp.tile([128, 2, 128], BF)
            nc.sync.dma_start(out=mqs, in_=mq.rearrange("p (t r) -> p t r", t=2))

            # SBUF residents across phases
            q_all = resp.tile([128, H, R], BF)        # [hd, h, row]
            kg_all = resp.tile([128, HKV, T], BF)     # [hd, g, key]
            v_all = resp.tile([128, NJT, KVD], BF)    # [key%128, kt, c]

            # rms-normalize PSUM tile [128,512] per token, rope, write bf16
            # to dst AP. For q, 1/sqrt(HD) and head gain fold into the scale.
            # Two pipeline stages: A (square + copy + sum-of-squares matmul)
            # releases the PSUM tile right away; B (sqrt/recip/broadcast/
            # rope) runs one step later so ACT's sqrt never sits in front
            # of the next tile's square in the ACT queue.
            def rms_stage_a(tmpp, ps, ssp):
                sq = tmpp.tile([128, 512], BF, tag="rr_sq")
                nc.scalar.square(sq, ps)
                pq = tmpp.tile([128, 512], F32, tag="rr_pq")
                nc.scalar.copy(pq, ps)
                ss = ssp.tile([1, 512], F32, tag="rr_ss")
                nc.tensor.matmul(ss, lhsT=ones, rhs=sq, start=True, stop=True)
                return pq, ss

            def rms_stage_b(tmpp, pq, ss, cs, isl, dst, gain_ap):
                sq_s = tmpp.tile([1, 512], F32, tag="rr_sqs")
                scl = tmpp.tile([1, 512], F32, tag="rr_scl")
                if gain_ap is not None:
                    nc.scalar.activation(sq_s, ss, AF.Sqrt, bias=epsq[0:1, 0:1])
                    nc.vector.reciprocal_approx_fast(scl, sq_s)
                    nc.vector.tensor_scalar_mul(scl, in0=scl, scalar1=gain_ap)
                else:
                    nc.scalar.activation(sq_s, ss, AF.Sqrt, bias=epsk[0:1, 0:1],
                                         scale=1.0 / HD)
                    nc.vector.reciprocal_approx_fast(scl, sq_s)
                sclb = tmpp.tile([128, 512], F32, tag="rr_sclb")
                nc.gpsimd.partition_broadcast(sclb, scl)
                qn = tmpp.tile([128, 512], F32, tag="rr_qn")
                nc.vector.tensor_mul(qn, pq, sclb)
                qnsw = tmpp.tile([128, 512], F32, tag="rr_qnsw")
                nc.sync.dma_start(out=qnsw[0:64], in_=qn[64:128])
                nc.sync.dma_start(out=qnsw[64:128], in_=qn[0:64])
                t12 = tmpp.tile([128, 512], BF, tag="rr_t12")
                nc.vector.tensor_mul(t12, qn, cs[:, 0, isl])
                t34 = tmpp.tile([128, 512], BF, tag="rr_t34")
                nc.vector.tensor_mul(t34, qnsw, cs[:, 1, isl])
                nc.vector.tensor_add(dst, t12, t34)

            # ---------------- Phase Q ----------------
            if True:
                pa = pb = None
                for hg in range(8):   # 2 heads per weight group
                    if hg == 0:
                        wqt = wq0
                    else:
                        wqt = wqp.tile([128, DT, 256], BF, tag="wq")
                        nc.sync.dma_start(out=wqt, in_=wq[:, hg])
                    prefetch_kv(hg)
                    for hh in range(2):
                        h = hg * 2 + hh
                        for ib in range(2):
                            isl = slice(ib * 512, (ib + 1) * 512)
                            ps = psp.tile([128, 512], F32, tag="ps")
                            for dt_ in range(DT):
                                nc.tensor.matmul(
                                    ps,
                                    lhsT=wqt[:, dt_, hh * HD:(hh + 1) * HD],
                                    rhs=xqt[:, dt_, isl],
                                    start=(dt_ == 0), stop=(dt_ == DT - 1))
                            if pa is not None:
                                pq, ss = rms_stage_a(tmpp, pa[0], ssp)
                                rms_stage_b(tmpp, pq, ss, *pa[1:])
                            pa = (ps, cq, isl,
                                  q_all[:, h, isl], qg[0:1, h:h + 1])
                pq, ss = rms_stage_a(tmpp, pa[0], ssp)
                rms_stage_b(tmpp, pq, ss, *pa[1:])
            cq_ctx.__exit__(None, None, None)
            wq_ctx.__exit__(None, None, None)
            xq_ctx.__exit__(None, None, None)

            # ---------------- Phase K/V ----------------
            xt_ctx = tc.tile_pool(name="xt", bufs=2)
            xtp = xt_ctx.__enter__()
            xt_tiles = {}

            def fetch_xt(jb, chunked=False):
                t_ = xtp.tile([128, DT, 512], BF, tag="xt")
                if chunked:  # first tile: let the K matmul chain start early
                    for lo, hi in ((0, 1), (1, 2), (2, 3), (3, 4), (4, 6),
                                   (6, 8), (8, 12), (12, DT)):
                        nc.sync.dma_start(out=t_[:, lo:hi, :],
                                          in_=xt[:, jb, lo:hi, :])
                else:
                    nc.sync.dma_start(out=t_, in_=xt[:, jb])
                xt_tiles[jb] = t_

            fetch_xt(0, chunked=True)
            fetch_xt(1)
            if True:
                pa = None

                def kv_advance(nxt):
                    nonlocal pa
                    if pa is not None:
                        if pa[0] == 'k':
                            _, ps, jsl, dst = pa
                            pq, ss = rms_stage_a(tmpp, ps, ssp)
                            rms_stage_b(tmpp, pq, ss, ck, jsl, dst, None)
                        else:
                            _, dst, psv = pa
                            nc.scalar.copy(dst, psv)
                    pa = nxt

                for jb in range(4):
                    jsl = slice(jb * 512, (jb + 1) * 512)
                    xtt = xt_tiles.pop(jb)
                    if jb + 2 < 4:
                        fetch_xt(jb + 2)
                    for g in range(HKV):
                        ps = psp.tile([128, 512], F32, tag="ps")
                        for dt_ in range(DT):
                            nc.tensor.matmul(
                                ps,
                                lhsT=wkt[:, dt_, g * HD:(g + 1) * HD],
                                rhs=xtt[:, dt_, :],
                                start=(dt_ == 0), stop=(dt_ == DT - 1))
                        kv_advance(('k', ps, jsl, kg_all[:, g, jsl]))
                    for jt in range(4):
                        psv = psp.tile([128, 512], F32, tag="ps")
                        for dt_ in range(DT):
                            nc.tensor.matmul(
                                psv,
                                lhsT=xtt[:, dt_, jt * 128:(jt + 1) * 128],
                                rhs=wvt[:, dt_, :],
                                start=(dt_ == 0), stop=(dt_ == DT - 1))
                        kv_advance(('v', v_all[:, jb * 4 + jt, :], psv))
                kv_advance(None)
            xt_ctx.__exit__(None, None, None)
            ss_ctx.__exit__(None, None, None)
            ps_ctx.__exit__(None, None, None)
            rms_ctx.__exit__(None, None, None)
            ck_ctx.__exit__(None, None, None)
            kvw_ctx.__exit__(None, None, None)

            # ---- Attention + proj (y and proj weights live here) ----
            with tc.tile_pool(name="res2", bufs=1) as res2p:
                # Prefetch all proj weights during attention (resident).
                wpt = res2p.tile([128, 4, DT, 512], BF)
                for og in range(4):
                    nc.sync.dma_start(out=wpt[:, og], in_=wp[:, og])
                y_all = res2p.tile([128, H, R], BF)   # [hd, h, row]

                # ---------------- Phase attention ----------------
                # One step = (head h, q-tile slot i). Steps are software-
                # pipelined: scores+exp+mask of step u are emitted before
                # the ys/normalize of step u-1, so the PE streams while
                # ACT exps the previous step's strips. The causal mask is
                # applied in place on GpSimd; the softmax denominator is
                # tree-folded per strip on DVE (bf16) so the PE only pays
                # one 128-row 1^T-matmul per strip instead of per key
                # tile.
                with tc.tile_pool(name="pts", bufs=4) as ptp, \
                     tc.tile_pool(name="fold", bufs=3) as ftp, \
                     tc.tile_pool(name="ntp", bufs=3) as ntp, \
                     tc.tile_pool(name="sc", bufs=2, space="PSUM") as scp, \
                     tc.tile_pool(name="ys", bufs=3, space="PSUM") as ysp:

                    def emit_scores(g, h, i):
                        """Scores + exp + in-place mask + one fold level
                        (w -> w/2 on DVE, bf16) for step (h, i)."""
                        e = EPROC[i]
                        tsl = slice(i * 128, (i + 1) * 128)
                        pts = []           # (pt_tile, fold_tile, w, kt_base)
                        kt_base = 0
                        for w in _strips(e):
                            sp = scp.tile([128, 8, 128], F32, tag="sc")
                            for k in range(w):
                                kt = kt_base + k
                                nc.tensor.matmul(
                                    sp[:, k, :],
                                    lhsT=kg_all[:, g, kt * 128:(kt + 1) * 128],
                                    rhs=q_all[:, h, tsl],
                                    start=True, stop=True)
                            pt = ptp.tile([128, 8, 128], BF, tag="pt")
                            # exp in two half-strip chunks: the ys chain's
                            # first matmuls only wait on the first half.
                            h1 = min(w, 4)
                            nc.scalar.activation(pt[:, 0:h1, :],
                                                 sp[:, 0:h1, :], AF.Exp)
                            if w > 4:
                                nc.scalar.activation(pt[:, 4:w, :],
                                                     sp[:, 4:w, :], AF.Exp)
                            if kt_base + w == e:   # strip has the last 2 kts
                                tl = (e - 2) - kt_base
                                nc.vector.tensor_mul(pt[:, tl:tl + 2, :],
                                                     pt[:, tl:tl + 2, :], mqs)
                            fh = ftp.tile([128, 4, 128], BF, tag="fh")
                            nc.vector.tensor_add(fh[:, 0:w // 2, :],
                                                 pt[:, 0:w // 2, :],
                                                 pt[:, w // 2:w, :])
                            pts.append((pt, fh, w, kt_base))
                            kt_base += w
                        return (g, h, i, e, pts)

                    def emit_consume(st):
                        g, h, i, e, pts = st
                        tsl = slice(i * 128, (i + 1) * 128)
                        ys = ysp.tile([128, 512], F32, tag="ys")
                        for pt, fh, w, kt_base in pts:
                            for k in range(w):
                                kt = kt_base + k
                                nc.tensor.matmul(
                                    ys[:, 0:128],
                                    lhsT=v_all[:, kt, g * HD:(g + 1) * HD],
                                    rhs=pt[:, k, :],
                                    start=(kt == 0), stop=(kt == e - 1))
                        nd = e // 2
                        di = 0
                        for pt, fh, w, kt_base in pts:
                            for j in range(w // 2):
                                nc.tensor.matmul(
                                    ys[0:1, 256:384], lhsT=ones,
                                    rhs=fh[:, j, :],
                                    start=(di == 0), stop=(di == nd - 1))
                                di += 1
                        rc = ntp.tile([1, 128], F32, tag="rc")
                        nc.vector.reciprocal_approx_fast(rc, ys[0:1, 256:384])
                        rcb = ntp.tile([128, 128], F32, tag="rcb")
                        nc.gpsimd.partition_broadcast(rcb, rc)
                        nc.vector.tensor_mul(y_all[:, h, tsl], ys[:, 0:128],
                                             rcb)

                    # Slots run in ascending causal extent (2,4,...,16): a
                    # step's scores+exp burst is then always paired with a
                    # same-or-bigger consume of the previous step, so the
                    # PE never outruns ACT at head boundaries, and the
                    # first steps only need the earliest K blocks.
                    prev = None
                    for g in range(HKV):
                        for hh in range(REP):
                            h = g * REP + hh
                            for i in reversed(range(8)):
                                st = emit_scores(g, h, i)
                                if prev is not None:
                                    emit_consume(prev)
                                prev = st
                    emit_consume(prev)

                # ---------------- Phase proj ----------------
                with tc.tile_pool(name="obp", bufs=2) as obp, \
                     tc.tile_pool(name="pso", bufs=2, space="PSUM") as psp:
                    for og in range(4):
                        for oo in range(4):
                            ot = og * 4 + oo
                            for ib in range(2):
                                isl = slice(ib * 512, (ib + 1) * 512)
                                ps = psp.tile([128, 512], F32, tag="o_ps")
                                for ct in range(DT):
                                    nc.tensor.matmul(
                                        ps,
                                        lhsT=wpt[:, og, ct,
                                                 oo * 128:(oo + 1) * 128],
                                        rhs=y_all[:, ct, isl],
                                        start=(ct == 0), stop=(ct == DT - 1))
                                ob = obp.tile([128, 512], F32, tag="ob")
                                nc.vector.tensor_copy(ob, ps)
                                nc.sync.dma_start(
                                    out=outT[ot * 128:(ot + 1) * 128, isl],
                                    in_=ob)

    nc.compile()
    return nc


def _rope_tables():
    inv = (1.0 / (np.float32(ROPE_BASE)
                  ** (np.arange(0, HD, 2, dtype=np.float32) / np.float32(HD))))
    t = np.arange(T, dtype=np.float32)
    freqs = np.outer(t, inv).astype(np.float32)          # [T, 64]
    c, si = np.cos(freqs).T, np.sin(freqs).T             # [64, T]
    # rows 0..63 twice for cos; +sin rows then -sin rows: with qn-halves
    # swapped this computes (q1*c + q2*s, q2*c - q1*s) in aligned DVE ops.
    cos_full = np.ascontiguousarray(np.concatenate([c, c], axis=0))
    sin_signed = np.ascontiguousarray(np.concatenate([si, -si], axis=0))
    return cos_full, sin_signed


def _proc_tiles(half):
    """q-tile (128-row block) indices in processing order: extent of slot
    i must be <= EPROC[i]."""
    return [e - 2 for e in EPROC] if half == 0 else [e - 1 for e in EPROC]


def _mask(half):
    """[128 key, 2, 128 row] {0,1} bf16 mask for the last 2 key tiles of
    every strip: half0 -> [tri, 0], half1 -> [1, tri]."""
    jj = np.arange(128)[:, None]
    rr = np.arange(128)[None, :]
    tri = (jj <= rr).astype(BF16)
    m = np.zeros((128, 2, 128), dtype=BF16)
    if half == 0:
        m[:, 0] = tri
    else:
        m[:, 0] = 1
        m[:, 1] = tri
    return np.ascontiguousarray(m.reshape(128, 256))


def _pdt(aT):
    """[dt*128, N] -> [128, dt, N] (partition-major, contiguous per part)."""
    d, n = aT.shape
    return np.ascontiguousarray(aT.reshape(d // 128, 128, n).transpose(1, 0, 2))


def kernel(**inputs):
    from concourse.bass_utils import run_bass_kernel_spmd

    x = np.ascontiguousarray(np.asarray(inputs["x"], dtype=np.float32))
    Wq = np.asarray(inputs["Wq"], dtype=np.float32)
    Wk = np.asarray(inputs["Wk"], dtype=np.float32)
    Wv = np.asarray(inputs["Wv"], dtype=np.float32)
    Wproj = np.asarray(inputs["Wproj"], dtype=np.float32)
    q_gain = np.ascontiguousarray(np.asarray(inputs["q_gain"], dtype=np.float32))

    if "nc" not in _CACHE:
        _CACHE["nc"] = _build()
    nc = _CACHE["nc"]

    def tb(a):  # transpose + bf16, contiguous
        return np.ascontiguousarray(a.T.astype(BF16))

    # wq: [128, dt, 2048] -> [128, hg=8, dt, 256] group-major
    wq_a = _pdt(tb(Wq)).reshape(128, DT, 8, 256).transpose(0, 2, 1, 3)
    wq_a = np.ascontiguousarray(wq_a)
    wk_a = _pdt(tb(Wk))
    wv_a = _pdt(tb(Wv))
    # wp: [128, ct, 2048] -> [128, og=4, ct, 512]
    wp_a = _pdt(tb(Wproj)).reshape(128, DT, 4, 512).transpose(0, 2, 1, 3)
    wp_a = np.ascontiguousarray(wp_a)
    cosT, sinT = _rope_tables()

    in_maps = []
    for c in range(8):
        b, half = divmod(c, 2)
        tiles = _proc_tiles(half)
        ridx = np.concatenate([np.arange(t * 128, (t + 1) * 128) for t in tiles])
        xb = x[b]
        # xt: [128, dt, 2048 tokens] -> [128, jb=4, dt, 512]
        xt_a = _pdt(tb(xb)).reshape(128, DT, 4, 512).transpose(0, 2, 1, 3)
        in_maps.append({
            "xq": _pdt(tb(xb[ridx])),
            "xt": np.ascontiguousarray(xt_a),
            "wq": wq_a, "wk": wk_a, "wv": wv_a, "wp": wp_a,
            "qgain": q_gain,
            "cosq": np.ascontiguousarray(cosT[:, ridx]),
            "sinq": np.ascontiguousarray(sinT[:, ridx]),
            "cosk": cosT, "sink": sinT,
            "mq": _mask(half),
        })

    res = run_bass_kernel_spmd(nc, in_maps, core_ids=list(range(8)),
                               tmpdir=os.environ.get("BASS_KERNEL_TMPDIR"))
    _CACHE["res"] = res

    out = np.empty((B, T, DIM), dtype=np.float32)
    for c in range(8):
        b, half = divmod(c, 2)
        oT = res.results[c]["outT"]
        for i, t in enumerate(_proc_tiles(half)):
            out[b, t * 128:(t + 1) * 128] = oT[:, i * 128:(i + 1) * 128].T
    return out


# revision 38
# speedup vs baseline: 1.0375x; 1.0375x over previous
"""Causal self-attention (GQA + RMSNorm + RoPE) Trainium2 Bass kernel.

Sharding: data-parallel over (batch, q-rows). 8 cores = 4 batches x 2 row
sets. Each core computes full K/V for its batch and 1024 q rows chosen as
8 x 128-row tiles: core half 0 takes even tiles, half 1 odd tiles. Tiles
are processed in descending causal-extent order so that a single
compile-time key-extent schedule E = (16,14,12,10,8,6,4,2) (in 128-key
tiles) is an upper bound for both halves: total scored coverage is 72
units/head vs 68 ideal causal, vs 96 dense-halves. No collectives.

All DRAM operands are pre-arranged on the host into [128-partition,
chunk, free] layouts so every load is one contiguous run per partition
(128 descriptors instead of thousands). K-phase weights prefetch during
phase Q; proj weights prefetch during attention.

On-chip layout is channel-major: scores are computed key-major
(S^T tile = K_tile^T.T @ Q^T) in [128,8,128] PSUM strips, exp'd in one
wide ACT instruction per strip, causal-masked in place by a {0,1}
multiply on only the last two key tiles (diagonal triangle + optional
padding), and consumed by per-q-tile accumulating ys (V^T @ P) matmul
chains. The softmax denominator does one DVE fold level per strip
(w -> w/2 pairwise adds, bf16 2x mode) so the PE only streams e/2
1^T-matmuls per step instead of e. Normalization (reciprocal_approx +
gpsimd partition broadcast) is applied to the y tile.

The QKV rms/rope epilogue copies each PSUM tile to SBUF with one ACT op
immediately after the matmul chain stops, so the PE's PSUM buffer is
released after two quick ACT reads instead of being held through the
whole serial sqrt/recip/broadcast/rope chain (which previously stalled
the PE ~4us per head). The whole attention pipeline is software-
pipelined one (head, q-tile) step deep. Q stays SBUF-resident between
phases. All matmul operands bf16 (fp32 accumulate); softmax/statistics
math fp32.
"""

import os
import sys

sys.path.insert(0, "/opt/trn_rl_repo")

import ml_dtypes
import numpy as np

DIM = 2048
H = 16
HKV = 4
HD = 128
REP = H // HKV
B = 4
T = 2048
R = 1024          # q rows per core
DT = DIM // 128   # 16 contraction tiles
NJT = T // 128    # 16 key tiles
KVD = HKV * HD    # 512
EPROC = (16, 14, 12, 10, 8, 6, 4, 2)  # key-tile extent per q-tile slot
ROPE_BASE = 10000.0
EPS = float(np.finfo(np.float32).eps)
BF16 = ml_dtypes.bfloat16

_CACHE = {}


def _strips(e):
    """Split an extent into PSUM-strip chunk widths (max 8 key tiles)."""
    out = [8] * (e // 8)
    if e % 8:
        out.append(e % 8)
    return out


def _build():
    """Build + compile the SPMD Bass program (once per process)."""
    from concourse import bacc
    import concourse.mybir as mybir
    import concourse.tile as tile

    F32 = mybir.dt.float32
    BF = mybir.dt.bfloat16
    AF = mybir.ActivationFunctionType

    nc = bacc.Bacc("TRN2", target_bir_lowering=False, debug=False)

    # All tensors pre-arranged host-side: partition dim first, contiguous
    # free bytes per partition for every dma slice taken below.
    xq = nc.dram_tensor("xq", [128, DT, R], BF, kind="ExternalInput")
    xt = nc.dram_tensor("xt", [128, 4, DT, 512], BF, kind="ExternalInput")
    wq = nc.dram_tensor("wq", [128, 8, DT, 256], BF, kind="ExternalInput")
    wk = nc.dram_tensor("wk", [128, DT, KVD], BF, kind="ExternalInput")
    wv = nc.dram_tensor("wv", [128, DT, KVD], BF, kind="ExternalInput")
    wp = nc.dram_tensor("wp", [128, 4, DT, 512], BF, kind="ExternalInput")
    qgain = nc.dram_tensor("qgain", [H], F32, kind="ExternalInput")
    cosq = nc.dram_tensor("cosq", [HD, R], F32, kind="ExternalInput")
    sinq = nc.dram_tensor("sinq", [HD, R], F32, kind="ExternalInput")
    cosk = nc.dram_tensor("cosk", [HD, T], F32, kind="ExternalInput")
    sink = nc.dram_tensor("sink", [HD, T], F32, kind="ExternalInput")
    # per-core {0,1} mask for the last two key tiles of every q-tile strip:
    # half0 -> [tri, 0], half1 -> [1, tri]  (key-major [key, 2, row])
    mq = nc.dram_tensor("mq", [128, 2 * 128], BF, kind="ExternalInput")
    outT = nc.dram_tensor("outT", [DIM, R], F32, kind="ExternalOutput")

    with tile.TileContext(nc) as tc:
        with tc.tile_pool(name="const", bufs=1) as constp, \
             tc.tile_pool(name="res", bufs=1) as resp:
            # Pool stack (LIFO release): kvw, ck, rms span Q+KV; xq/wq/cq
            # are Q-only and sit on top so they can be released after Q.
            kvw_ctx = tc.tile_pool(name="kvw", bufs=1)
            kvwp = kvw_ctx.__enter__()
            ck_ctx = tc.tile_pool(name="ckp", bufs=1)
            ckp = ck_ctx.__enter__()
            rms_ctx = tc.tile_pool(name="rms", bufs=2)
            tmpp = rms_ctx.__enter__()
            ps_ctx = tc.tile_pool(name="psqkv", bufs=4, space="PSUM")
            psp = ps_ctx.__enter__()
            ss_ctx = tc.tile_pool(name="ssqkv", bufs=4, space="PSUM")
            ssp = ss_ctx.__enter__()
            xq_ctx = tc.tile_pool(name="xq", bufs=1)
            xqp = xq_ctx.__enter__()
            wq_ctx = tc.tile_pool(name="wq", bufs=2)
            wqp = wq_ctx.__enter__()
            cq_ctx = tc.tile_pool(name="cq", bufs=1)
            cqp = cq_ctx.__enter__()

            # ---- Phase-Q critical loads first (in Sync program order) ----
            # Leading chunks are small so the first matmul's deps land fast.
            xqt = xqp.tile([128, DT, R], BF)
            wq0 = wqp.tile([128, DT, 256], BF, tag="wq")
            for lo, hi in ((0, 2), (2, 4)):
                nc.sync.dma_start(out=xqt[:, lo:hi, :], in_=xq[:, lo:hi, :])
                nc.sync.dma_start(out=wq0[:, lo:hi, :], in_=wq[:, 0, lo:hi, :])
            nc.sync.dma_start(out=wq0[:, 4:DT, :], in_=wq[:, 0, 4:DT, :])
            cq = cqp.tile([128, 2, R], F32)
            nc.sync.dma_start(out=cq[:, 0], in_=cosq[:, :])
            nc.sync.dma_start(out=cq[:, 1], in_=sinq[:, :])
            for ch in range(1, 4):
                nc.sync.dma_start(out=xqt[:, ch * 4:(ch + 1) * 4, :],
                                  in_=xq[:, ch * 4:(ch + 1) * 4, :])

            # K/V-phase weights prefetch: tiles allocated here, dma_starts
            # interleaved into the phase-Q loop so queue order stays behind
            # the urgent per-group wq loads.
            wkt = kvwp.tile([128, DT, KVD], BF)
            wvt = kvwp.tile([128, DT, KVD], BF)
            ck = ckp.tile([128, 2, T], F32)

            def prefetch_kv(hg):
                if hg == 4:
                    nc.sync.dma_start(out=wkt, in_=wk[:, :, :])
                elif hg == 5:
                    nc.sync.dma_start(out=wvt, in_=wv[:, :, :])
                elif hg == 6:
                    nc.sync.dma_start(out=ck[:, 0], in_=cosk[:, :])
                    nc.sync.dma_start(out=ck[:, 1], in_=sink[:, :])

            # ---- Constants ----
            ones = constp.tile([128, 1], BF)
            nc.vector.memset(ones, 1.0)
            qg = constp.tile([1, H], F32)
            nc.sync.dma_start(out=qg, in_=qgain[None, :])
            epsq = constp.tile([1, 1], F32)
            nc.vector.memset(epsq, EPS * HD)   # q scale: 1/sqrt(ssum + HD*eps)
            epsk = constp.tile([1, 1], F32)
            nc.vector.memset(epsk, EPS)        # k scale: rsqrt(ssum/HD + eps)
            mqs = constp.tile([128, 2, 128], BF)
            nc.sync.dma_start(out=mqs, in_=mq.rearrange("p (t r) -> p t r", t=2))

            # SBUF residents across phases
            q_all = resp.tile([128, H, R], BF)        # [hd, h, row]
            kg_all = resp.tile([128, HKV, T], BF)     # [hd, g, key]
            v_all = resp.tile([128, NJT, KVD], BF)    # [key%128, kt, c]

            # rms-normalize PSUM tile [128,512] per token, rope, write bf16
            # to dst AP. For q, 1/sqrt(HD) and head gain fold into the scale.
            # Two pipeline stages: A (square + copy + sum-of-squares matmul)
            # releases the PSUM tile right away; B (sqrt/recip/broadcast/
            # rope) runs one step later so ACT's sqrt never sits in front
            # of the next tile's square in the ACT queue.
            def rms_stage_a(tmpp, ps, ssp):
                sq = tmpp.tile([128, 512], BF, tag="rr_sq")
                nc.scalar.square(sq, ps)
                pq = tmpp.tile([128, 512], F32, tag="rr_pq")
                nc.scalar.copy(pq, ps)
                ss = ssp.tile([1, 512], F32, tag="rr_ss")
                nc.tensor.matmul(ss, lhsT=ones, rhs=sq, start=True, stop=True)
                return pq, ss

            def rms_stage_b(tmpp, pq, ss, cs, isl, dst, gain_ap):
                sq_s = tmpp.tile([1, 512], F32, tag="rr_sqs")
                scl = tmpp.tile([1, 512], F32, tag="rr_scl")
                if gain_ap is not None:
                    nc.scalar.activation(sq_s, ss, AF.Sqrt, bias=epsq[0:1, 0:1])
                    nc.vector.reciprocal_approx_fast(scl, sq_s)
                    nc.vector.tensor_scalar_mul(scl, in0=scl, scalar1=gain_ap)
                else:
                    nc.scalar.activation(sq_s, ss, AF.Sqrt, bias=epsk[0:1, 0:1],
                                         scale=1.0 / HD)
                    nc.vector.reciprocal_approx_fast(scl, sq_s)
                sclb = tmpp.tile([128, 512], F32, tag="rr_sclb")
                nc.gpsimd.partition_broadcast(sclb, scl)
                qn = tmpp.tile([128, 512], F32, tag="rr_qn")
                nc.vector.tensor_mul(qn, pq, sclb)
                qnsw = tmpp.tile([128, 512], F32, tag="rr_qnsw")
                nc.sync.dma_start(out=qnsw[0:64], in_=qn[64:128])
                nc.sync.dma_start(out=qnsw[64:128], in_=qn[0:64])
                t12 = tmpp.tile([128, 512], BF, tag="rr_t12")
                nc.vector.tensor_mul(t12, qn, cs[:, 0, isl])
                t34 = tmpp.tile([128, 512], BF, tag="rr_t34")
                nc.vector.tensor_mul(t34, qnsw, cs[:, 1, isl])
                nc.vector.tensor_add(dst, t12, t34)

            # ---------------- Phase Q ----------------
            if True:
                pa = pb = None
                for hg in range(8):   # 2 heads per weight group
                    if hg == 0:
                        wqt = wq0
                    else:
                        wqt = wqp.tile([128, DT, 256], BF, tag="wq")
                        nc.sync.dma_start(out=wqt, in_=wq[:, hg])
                    prefetch_kv(hg)
                    for hh in range(2):
                        h = hg * 2 + hh
                        for ib in range(2):
                            isl = slice(ib * 512, (ib + 1) * 512)
                            ps = psp.tile([128, 512], F32, tag="ps")
                            for dt_ in range(DT):
                                nc.tensor.matmul(
                                    ps,
                                    lhsT=wqt[:, dt_, hh * HD:(hh + 1) * HD],
                                    rhs=xqt[:, dt_, isl],
                                    start=(dt_ == 0), stop=(dt_ == DT - 1))
                            if pa is not None:
                                pq, ss = rms_stage_a(tmpp, pa[0], ssp)
                                rms_stage_b(tmpp, pq, ss, *pa[1:])
                            pa = (ps, cq, isl,
                                  q_all[:, h, isl], qg[0:1, h:h + 1])
                pq, ss = rms_stage_a(tmpp, pa[0], ssp)
                rms_stage_b(tmpp, pq, ss, *pa[1:])
            cq_ctx.__exit__(None, None, None)
            wq_ctx.__exit__(None, None, None)
            xq_ctx.__exit__(None, None, None)

            # ---------------- Phase K/V ----------------
            xt_ctx = tc.tile_pool(name="xt", bufs=2)
            xtp = xt_ctx.__enter__()
            xt_tiles = {}

            def fetch_xt(jb, chunked=False):
                t_ = xtp.tile([128, DT, 512], BF, tag="xt")
                if chunked:  # first tile: let the K matmul chain start early
                    for lo, hi in ((0, 1), (1, 2), (2, 3), (3, 4), (4, 6),
                                   (6, 8), (8, 12), (12, DT)):
                        nc.sync.dma_start(out=t_[:, lo:hi, :],
                                          in_=xt[:, jb, lo:hi, :])
                else:
                    nc.sync.dma_start(out=t_, in_=xt[:, jb])
                xt_tiles[jb] = t_

            fetch_xt(0, chunked=True)
            fetch_xt(1)
            if True:
                pa = None

                def kv_advance(nxt):
                    nonlocal pa
                    if pa is not None:
                        if pa[0] == 'k':
                            _, ps, jsl, dst = pa
                            pq, ss = rms_stage_a(tmpp, ps, ssp)
                            rms_stage_b(tmpp, pq, ss, ck, jsl, dst, None)
                        else:
                            _, dst, psv = pa
                            nc.scalar.copy(dst, psv)
                    pa = nxt

                for jb in range(4):
                    jsl = slice(jb * 512, (jb + 1) * 512)
                    xtt = xt_tiles.pop(jb)
                    if jb + 2 < 4:
                        fetch_xt(jb + 2)
                    for g in range(HKV):
                        ps = psp.tile([128, 512], F32, tag="ps")
                        for dt_ in range(DT):
                            nc.tensor.matmul(
                                ps,
                                lhsT=wkt[:, dt_, g * HD:(g + 1) * HD],
                                rhs=xtt[:, dt_, :],
                                start=(dt_ == 0), stop=(dt_ == DT - 1))
                        kv_advance(('k', ps, jsl, kg_all[:, g, jsl]))
                    for jt in range(4):
                        psv = psp.tile([128, 512], F32, tag="ps")
                        for dt_ in range(DT):
                            nc.tensor.matmul(
                                psv,
                                lhsT=xtt[:, dt_, jt * 128:(jt + 1) * 128],
                                rhs=wvt[:, dt_, :],
                                start=(dt_ == 0), stop=(dt_ == DT - 1))
                        kv_advance(('v', v_all[:, jb * 4 + jt, :], psv))
                kv_advance(None)
            xt_ctx.__exit__(None, None, None)
            ss_ctx.__exit__(None, None, None)
            ps_ctx.__exit__(None, None, None)
            rms_ctx.__exit__(None, None, None)
            ck_ctx.__exit__(None, None, None)
            kvw_ctx.__exit__(None, None, None)

            # ---- Attention + proj (y and proj weights live here) ----
            with tc.tile_pool(name="res2", bufs=1) as res2p:
                # Prefetch all proj weights during attention (resident).
                wpt = res2p.tile([128, 4, DT, 512], BF)
                for og in range(4):
                    nc.sync.dma_start(out=wpt[:, og], in_=wp[:, og])
                y_all = res2p.tile([128, H, R], BF)   # [hd, h, row]

                # ---------------- Phase attention ----------------
                # One step = (head h, q-tile slot i). Steps are software-
                # pipelined: scores+exp+mask of step u are emitted before
                # the ys/normalize of step u-1, so the PE streams while
                # ACT exps the previous step's strips. The causal mask is
                # applied in place on GpSimd; the softmax denominator is
                # tree-folded per strip on DVE (bf16) so the PE only pays
                # one 128-row 1^T-matmul per strip instead of per key
                # tile.
                with tc.tile_pool(name="pts", bufs=4) as ptp, \
                     tc.tile_pool(name="fold", bufs=3) as ftp, \
                     tc.tile_pool(name="ntp", bufs=3) as ntp, \
                     tc.tile_pool(name="sc", bufs=2, space="PSUM") as scp, \
                     tc.tile_pool(name="ys", bufs=3, space="PSUM") as ysp:

                    def emit_scores(g, h, i):
                        """Scores + exp + in-place mask + one fold level
                        (w -> w/2 on DVE, bf16) for step (h, i)."""
                        e = EPROC[i]
                        tsl = slice(i * 128, (i + 1) * 128)
                        pts = []           # (pt_tile, fold_tile, w, kt_base)
                        kt_base = 0
                        for w in _strips(e):
                            sp = scp.tile([128, 8, 128], F32, tag="sc")
                            for k in range(w):
                                kt = kt_base + k
                                nc.tensor.matmul(
                                    sp[:, k, :],
                                    lhsT=kg_all[:, g, kt * 128:(kt + 1) * 128],
                                    rhs=q_all[:, h, tsl],
                                    start=True, stop=True)
                            pt = ptp.tile([128, 8, 128], BF, tag="pt")
                            nc.scalar.activation(pt[:, 0:w, :], sp[:, 0:w, :],
                                                 AF.Exp)
                            if kt_base + w == e:   # strip has the last 2 kts
                                tl = (e - 2) - kt_base
                                nc.vector.tensor_mul(pt[:, tl:tl + 2, :],
                                                     pt[:, tl:tl + 2, :], mqs)
                            fh = ftp.tile([128, 4, 128], BF, tag="fh")
                            nc.vector.tensor_add(fh[:, 0:w // 2, :],
                                                 pt[:, 0:w // 2, :],
                                                 pt[:, w // 2:w, :])
                            pts.append((pt, fh, w, kt_base))
                            kt_base += w
                        return (g, h, i, e, pts)

                    def emit_consume(st):
                        g, h, i, e, pts = st
                        tsl = slice(i * 128, (i + 1) * 128)
                        ys = ysp.tile([128, 512], F32, tag="ys")
                        for pt, fh, w, kt_base in pts:
                            for k in range(w):
                                kt = kt_base + k
                                nc.tensor.matmul(
                                    ys[:, 0:128],
                                    lhsT=v_all[:, kt, g * HD:(g + 1) * HD],
                                    rhs=pt[:, k, :],
                                    start=(kt == 0), stop=(kt == e - 1))
                        nd = e // 2
                        di = 0
                        for pt, fh, w, kt_base in pts:
                            for j in range(w // 2):
                                nc.tensor.matmul(
                                    ys[0:1, 256:384], lhsT=ones,
                                    rhs=fh[:, j, :],
                                    start=(di == 0), stop=(di == nd - 1))
                                di += 1
                        rc = ntp.tile([1, 128], F32, tag="rc")
                        nc.vector.reciprocal_approx_fast(rc, ys[0:1, 256:384])
                        rcb = ntp.tile([128, 128], F32, tag="rcb")
                        nc.gpsimd.partition_broadcast(rcb, rc)
                        nc.vector.tensor_mul(y_all[:, h, tsl], ys[:, 0:128],
                                             rcb)

                    # Slots run in ascending causal extent (2,4,...,16): a
                    # step's scores+exp burst is then always paired with a
                    # same-or-bigger consume of the previous step, so the
                    # PE never outruns ACT at head boundaries, and the
                    # first steps only need the earliest K blocks.
                    prev = None
                    for g in range(HKV):
                        for hh in range(REP):
                            h = g * REP + hh
                            for i in reversed(range(8)):
                                st = emit_scores(g, h, i)
                                if prev is not None:
                                    emit_consume(prev)
                                prev = st
                    emit_consume(prev)

                # ---------------- Phase proj ----------------
                with tc.tile_pool(name="obp", bufs=2) as obp, \
                     tc.tile_pool(name="pso", bufs=2, space="PSUM") as psp:
                    for og in range(4):
                        for oo in range(4):
                            ot = og * 4 + oo
                            for ib in range(2):
                                isl = slice(ib * 512, (ib + 1) * 512)
                                ps = psp.tile([128, 512], F32, tag="o_ps")
                                for ct in range(DT):
                                    nc.tensor.matmul(
                                        ps,
                                        lhsT=wpt[:, og, ct,
                                                 oo * 128:(oo + 1) * 128],
                                        rhs=y_all[:, ct, isl],
                                        start=(ct == 0), stop=(ct == DT - 1))
                                ob = obp.tile([128, 512], F32, tag="ob")
                                nc.vector.tensor_copy(ob, ps)
                                nc.sync.dma_start(
                                    out=outT[ot * 128:(ot + 1) * 128, isl],
                                    in_=ob)

    nc.compile()
    return nc


def _rope_tables():
    inv = (1.0 / (np.float32(ROPE_BASE)
                  ** (np.arange(0, HD, 2, dtype=np.float32) / np.float32(HD))))
    t = np.arange(T, dtype=np.float32)
    freqs = np.outer(t, inv).astype(np.float32)          # [T, 64]
    c, si = np.cos(freqs).T, np.sin(freqs).T             # [64, T]
    # rows 0..63 twice for cos; +sin rows then -sin rows: with qn-halves
    # swapped this computes (q1*c + q2*s, q2*c - q1*s) in aligned DVE ops.
    cos_full = np.ascontiguousarray(np.concatenate([c, c], axis=0))
    sin_signed = np.ascontiguousarray(np.concatenate([si, -si], axis=0))
    return cos_full, sin_signed


def _proc_tiles(half):
    """q-tile (128-row block) indices in processing order: extent of slot
    i must be <= EPROC[i]."""
    return [e - 2 for e in EPROC] if half == 0 else [e - 1 for e in EPROC]


def _mask(half):
    """[128 key, 2, 128 row] {0,1} bf16 mask for the last 2 key tiles of
    every strip: half0 -> [tri, 0], half1 -> [1, tri]."""
    jj = np.arange(128)[:, None]
    rr = np.arange(128)[None, :]
    tri = (jj <= rr).astype(BF16)
    m = np.zeros((128, 2, 128), dtype=BF16)
    if half == 0:
        m[:, 0] = tri
    else:
        m[:, 0] = 1
        m[:, 1] = tri
    return np.ascontiguousarray(m.reshape(128, 256))


def _pdt(aT):
    """[dt*128, N] -> [128, dt, N] (partition-major, contiguous per part)."""
    d, n = aT.shape
    return np.ascontiguousarray(aT.reshape(d // 128, 128, n).transpose(1, 0, 2))


def kernel(**inputs):
    from concourse.bass_utils import run_bass_kernel_spmd

    x = np.ascontiguousarray(np.asarray(inputs["x"], dtype=np.float32))
    Wq = np.asarray(inputs["Wq"], dtype=np.float32)
    Wk = np.asarray(inputs["Wk"], dtype=np.float32)
    Wv = np.asarray(inputs["Wv"], dtype=np.float32)
    Wproj = np.asarray(inputs["Wproj"], dtype=np.float32)
    q_gain = np.ascontiguousarray(np.asarray(inputs["q_gain"], dtype=np.float32))

    if "nc" not in _CACHE:
        _CACHE["nc"] = _build()
    nc = _CACHE["nc"]

    def tb(a):  # transpose + bf16, contiguous
        return np.ascontiguousarray(a.T.astype(BF16))

    # wq: [128, dt, 2048] -> [128, hg=8, dt, 256] group-major
    wq_a = _pdt(tb(Wq)).reshape(128, DT, 8, 256).transpose(0, 2, 1, 3)
    wq_a = np.ascontiguousarray(wq_a)
    wk_a = _pdt(tb(Wk))
    wv_a = _pdt(tb(Wv))
    # wp: [128, ct, 2048] -> [128, og=4, ct, 512]
    wp_a = _pdt(tb(Wproj)).reshape(128, DT, 4, 512).transpose(0, 2, 1, 3)
    wp_a = np.ascontiguousarray(wp_a)
    cosT, sinT = _rope_tables()

    in_maps = []
    for c in range(8):
        b, half = divmod(c, 2)
        tiles = _proc_tiles(half)
        ridx = np.concatenate([np.arange(t * 128, (t + 1) * 128) for t in tiles])
        xb = x[b]
        # xt: [128, dt, 2048 tokens] -> [128, jb=4, dt, 512]
        xt_a = _pdt(tb(xb)).reshape(128, DT, 4, 512).transpose(0, 2, 1, 3)
        in_maps.append({
            "xq": _pdt(tb(xb[ridx])),
            "xt": np.ascontiguousarray(xt_a),
            "wq": wq_a, "wk": wk_a, "wv": wv_a, "wp": wp_a,
            "qgain": q_gain,
            "cosq": np.ascontiguousarray(cosT[:, ridx]),
            "sinq": np.ascontiguousarray(sinT[:, ridx]),
            "cosk": cosT, "sink": sinT,
            "mq": _mask(half),
        })

    res = run_bass_kernel_spmd(nc, in_maps, core_ids=list(range(8)),
                               tmpdir=os.environ.get("BASS_KERNEL_TMPDIR"))
    _CACHE["res"] = res

    out = np.empty((B, T, DIM), dtype=np.float32)
    for c in range(8):
        b, half = divmod(c, 2)
        oT = res.results[c]["outT"]
        for i, t in enumerate(_proc_tiles(half)):
            out[b, t * 128:(t + 1) * 128] = oT[:, i * 128:(i + 1) * 128].T
    return out


# revision 43
# speedup vs baseline: 1.0409x; 1.0033x over previous
"""Causal self-attention (GQA + RMSNorm + RoPE) Trainium2 Bass kernel.

Sharding: data-parallel over (batch, q-rows). 8 cores = 4 batches x 2 row
sets. Each core computes full K/V for its batch and 1024 q rows chosen as
8 x 128-row tiles: core half 0 takes even tiles, half 1 odd tiles. Tiles
are processed in descending causal-extent order so that a single
compile-time key-extent schedule E = (16,14,12,10,8,6,4,2) (in 128-key
tiles) is an upper bound for both halves: total scored coverage is 72
units/head vs 68 ideal causal, vs 96 dense-halves. No collectives.

All DRAM operands are pre-arranged on the host into [128-partition,
chunk, free] layouts so every load is one contiguous run per partition
(128 descriptors instead of thousands). K-phase weights prefetch during
phase Q; proj weights prefetch during attention.

On-chip layout is channel-major: scores are computed key-major
(S^T tile = K_tile^T.T @ Q^T) in [128,8,128] PSUM strips, exp'd in one
wide ACT instruction per strip, causal-masked in place by a {0,1}
multiply on only the last two key tiles (diagonal triangle + optional
padding), and consumed by per-q-tile accumulating ys (V^T @ P) matmul
chains. The softmax denominator does one DVE fold level per strip
(w -> w/2 pairwise adds, bf16 2x mode) so the PE only streams e/2
1^T-matmuls per step instead of e. Normalization (reciprocal_approx +
gpsimd partition broadcast) is applied to the y tile.

The QKV rms/rope epilogue copies each PSUM tile to SBUF with one ACT op
immediately after the matmul chain stops, so the PE's PSUM buffer is
released after two quick ACT reads instead of being held through the
whole serial sqrt/recip/broadcast/rope chain (which previously stalled
the PE ~4us per head). The whole attention pipeline is software-
pipelined one (head, q-tile) step deep. Q stays SBUF-resident between
phases. All matmul operands bf16 (fp32 accumulate); softmax/statistics
math fp32.
"""

import os
import sys

sys.path.insert(0, "/opt/trn_rl_repo")

import ml_dtypes
import numpy as np

DIM = 2048
H = 16
HKV = 4
HD = 128
REP = H // HKV
B = 4
T = 2048
R = 1024          # q rows per core
DT = DIM // 128   # 16 contraction tiles
NJT = T // 128    # 16 key tiles
KVD = HKV * HD    # 512
EPROC = (16, 14, 12, 10, 8, 6, 4, 2)  # key-tile extent per q-tile slot
ROPE_BASE = 10000.0
EPS = float(np.finfo(np.float32).eps)
BF16 = ml_dtypes.bfloat16

_CACHE = {}


def _strips(e):
    """Split an extent into PSUM-strip chunk widths (max 8 key tiles)."""
    out = [8] * (e // 8)
    if e % 8:
        out.append(e % 8)
    return out


def _build():
    """Build + compile the SPMD Bass program (once per process)."""
    from concourse import bacc
    import concourse.mybir as mybir
    import concourse.tile as tile

    F32 = mybir.dt.float32
    BF = mybir.dt.bfloat16
    AF = mybir.ActivationFunctionType

    nc = bacc.Bacc("TRN2", target_bir_lowering=False, debug=False)

    # All tensors pre-arranged host-side: partition dim first, contiguous
    # free bytes per partition for every dma slice taken below.
    xq = nc.dram_tensor("xq", [128, DT, R], BF, kind="ExternalInput")
    xt = nc.dram_tensor("xt", [128, 4, DT, 512], BF, kind="ExternalInput")
    wq = nc.dram_tensor("wq", [128, 8, DT, 256], BF, kind="ExternalInput")
    wk = nc.dram_tensor("wk", [128, DT, KVD], BF, kind="ExternalInput")
    wv = nc.dram_tensor("wv", [128, DT, KVD], BF, kind="ExternalInput")
    wp = nc.dram_tensor("wp", [128, 4, DT, 512], BF, kind="ExternalInput")
    qgain = nc.dram_tensor("qgain", [H], F32, kind="ExternalInput")
    cosq = nc.dram_tensor("cosq", [HD, R], F32, kind="ExternalInput")
    sinq = nc.dram_tensor("sinq", [HD, R], F32, kind="ExternalInput")
    cosk = nc.dram_tensor("cosk", [HD, T], F32, kind="ExternalInput")
    sink = nc.dram_tensor("sink", [HD, T], F32, kind="ExternalInput")
    # per-core {0,1} mask for the last two key tiles of every q-tile strip:
    # half0 -> [tri, 0], half1 -> [1, tri]  (key-major [key, 2, row])
    mq = nc.dram_tensor("mq", [128, 2 * 128], BF, kind="ExternalInput")
    outT = nc.dram_tensor("outT", [DIM, R], F32, kind="ExternalOutput")

    with tile.TileContext(nc) as tc:
        with tc.tile_pool(name="const", bufs=1) as constp, \
             tc.tile_pool(name="res", bufs=1) as resp:
            # Pool stack (LIFO release): kvw, ck, rms span Q+KV; xq/wq/cq
            # are Q-only and sit on top so they can be released after Q.
            kvw_ctx = tc.tile_pool(name="kvw", bufs=1)
            kvwp = kvw_ctx.__enter__()
            rms_ctx = tc.tile_pool(name="rms", bufs=2)
            tmpp = rms_ctx.__enter__()
            ps_ctx = tc.tile_pool(name="psqkv", bufs=4, space="PSUM")
            psp = ps_ctx.__enter__()
            ss_ctx = tc.tile_pool(name="ssqkv", bufs=4, space="PSUM")
            ssp = ss_ctx.__enter__()
            xt0_ctx = tc.tile_pool(name="xt0", bufs=1)
            xt0p = xt0_ctx.__enter__()
            xq_ctx = tc.tile_pool(name="xq", bufs=1)
            xqp = xq_ctx.__enter__()
            wq_ctx = tc.tile_pool(name="wq", bufs=2)
            wqp = wq_ctx.__enter__()
            cq_ctx = tc.tile_pool(name="cq", bufs=1)
            cqp = cq_ctx.__enter__()

            # ---- Phase-Q critical loads first (in Sync program order) ----
            # Leading chunks are small so the first matmul's deps land fast.
            xqt = xqp.tile([128, DT, R], BF)
            wq0 = wqp.tile([128, DT, 256], BF, tag="wq")
            for lo, hi in ((0, 2), (2, 4)):
                nc.sync.dma_start(out=xqt[:, lo:hi, :], in_=xq[:, lo:hi, :])
                nc.sync.dma_start(out=wq0[:, lo:hi, :], in_=wq[:, 0, lo:hi, :])
            nc.sync.dma_start(out=wq0[:, 4:DT, :], in_=wq[:, 0, 4:DT, :])
            cq = cqp.tile([128, 2, R], F32)
            nc.sync.dma_start(out=cq[:, 0], in_=cosq[:, :])
            nc.sync.dma_start(out=cq[:, 1], in_=sinq[:, :])
            for ch in range(1, 4):
                nc.sync.dma_start(out=xqt[:, ch * 4:(ch + 1) * 4, :],
                                  in_=xq[:, ch * 4:(ch + 1) * 4, :])

            # K/V-phase weights + first-token-block prefetch: tiles
            # allocated here, dma_starts interleaved into the phase-Q loop
            # so queue order stays behind the urgent per-group wq loads.
            wkt = kvwp.tile([128, DT, KVD], BF)
            wvt = kvwp.tile([128, DT, KVD], BF)
            xt0 = xt0p.tile([128, DT, 512], BF)

            def prefetch_kv(hg):
                if hg == 4:
                    nc.sync.dma_start(out=wkt, in_=wk[:, :, :])
                elif hg == 5:
                    nc.sync.dma_start(out=wvt, in_=wv[:, :, :])
                elif hg == 6:
                    nc.sync.dma_start(out=xt0, in_=xt[:, 0])

            # ---- Constants ----
            ones = constp.tile([128, 1], BF)
            nc.vector.memset(ones, 1.0)
            qg = constp.tile([1, H], F32)
            nc.sync.dma_start(out=qg, in_=qgain[None, :])
            epsq = constp.tile([1, 1], F32)
            nc.vector.memset(epsq, EPS * HD)   # q scale: 1/sqrt(ssum + HD*eps)
            epsk = constp.tile([1, 1], F32)
            nc.vector.memset(epsk, EPS)        # k scale: rsqrt(ssum/HD + eps)
            mqs = constp.tile([128, 2, 128], BF)
            nc.sync.dma_start(out=mqs, in_=mq.rearrange("p (t r) -> p t r", t=2))

            # SBUF residents across phases
            q_all = resp.tile([128, H, R], BF)        # [hd, h, row]
            kg_all = resp.tile([128, HKV, T], BF)     # [hd, g, key]
            v_all = resp.tile([128, NJT, KVD], BF)    # [key%128, kt, c]

            # rms-normalize PSUM tile [128,512] per token, rope, write bf16
            # to dst AP. For q, 1/sqrt(HD) and head gain fold into the scale.
            # Two pipeline stages: A (square + copy + sum-of-squares matmul)
            # releases the PSUM tile right away; B (sqrt/recip/broadcast/
            # rope) runs one step later so ACT's sqrt never sits in front
            # of the next tile's square in the ACT queue.
            def rms_stage_a(tmpp, ps, ssp):
                sq = tmpp.tile([128, 512], BF, tag="rr_sq")
                nc.scalar.square(sq, ps)
                pq = tmpp.tile([128, 512], F32, tag="rr_pq")
                nc.scalar.copy(pq, ps)
                ss = ssp.tile([1, 512], F32, tag="rr_ss")
                nc.tensor.matmul(ss, lhsT=ones, rhs=sq, start=True, stop=True)
                return pq, ss

            def rms_stage_b(tmpp, pq, ss, cs, isl, dst, gain_ap):
                sq_s = tmpp.tile([1, 512], F32, tag="rr_sqs")
                scl = tmpp.tile([1, 512], F32, tag="rr_scl")
                if gain_ap is not None:
                    nc.scalar.activation(sq_s, ss, AF.Sqrt, bias=epsq[0:1, 0:1])
                    nc.vector.reciprocal_approx_fast(scl, sq_s)
                    nc.vector.tensor_scalar_mul(scl, in0=scl, scalar1=gain_ap)
                else:
                    nc.scalar.activation(sq_s, ss, AF.Sqrt, bias=epsk[0:1, 0:1],
                                         scale=1.0 / HD)
                    nc.vector.reciprocal_approx_fast(scl, sq_s)
                sclb = tmpp.tile([128, 512], F32, tag="rr_sclb")
                nc.gpsimd.partition_broadcast(sclb, scl)
                qn = tmpp.tile([128, 512], F32, tag="rr_qn")
                nc.vector.tensor_mul(qn, pq, sclb)
                qnsw = tmpp.tile([128, 512], F32, tag="rr_qnsw")
                nc.sync.dma_start(out=qnsw[0:64], in_=qn[64:128])
                nc.sync.dma_start(out=qnsw[64:128], in_=qn[0:64])
                t12 = tmpp.tile([128, 512], BF, tag="rr_t12")
                nc.vector.tensor_mul(t12, qn, cs[:, 0, isl])
                t34 = tmpp.tile([128, 512], BF, tag="rr_t34")
                nc.vector.tensor_mul(t34, qnsw, cs[:, 1, isl])
                nc.vector.tensor_add(dst, t12, t34)

            # ---------------- Phase Q ----------------
            if True:
                pa = pb = None
                for hg in range(8):   # 2 heads per weight group
                    if hg == 0:
                        wqt = wq0
                    else:
                        wqt = wqp.tile([128, DT, 256], BF, tag="wq")
                        nc.sync.dma_start(out=wqt, in_=wq[:, hg])
                    prefetch_kv(hg)
                    for hh in range(2):
                        h = hg * 2 + hh
                        for ib in range(2):
                            isl = slice(ib * 512, (ib + 1) * 512)
                            ps = psp.tile([128, 512], F32, tag="ps")
                            for dt_ in range(DT):
                                nc.tensor.matmul(
                                    ps,
                                    lhsT=wqt[:, dt_, hh * HD:(hh + 1) * HD],
                                    rhs=xqt[:, dt_, isl],
                                    start=(dt_ == 0), stop=(dt_ == DT - 1))
                            if pa is not None:
                                pq, ss = rms_stage_a(tmpp, pa[0], ssp)
                                rms_stage_b(tmpp, pq, ss, *pa[1:])
                            pa = (ps, cq, isl,
                                  q_all[:, h, isl], qg[0:1, h:h + 1])
                pq, ss = rms_stage_a(tmpp, pa[0], ssp)
                rms_stage_b(tmpp, pq, ss, *pa[1:])
            cq_ctx.__exit__(None, None, None)
            wq_ctx.__exit__(None, None, None)
            xq_ctx.__exit__(None, None, None)

            # ---------------- Phase K/V ----------------
            ck_ctx = tc.tile_pool(name="ckp", bufs=1)
            ckp = ck_ctx.__enter__()
            ck = ckp.tile([128, 2, T], F32)
            nc.sync.dma_start(out=ck[:, 0], in_=cosk[:, :])
            nc.sync.dma_start(out=ck[:, 1], in_=sink[:, :])

            xt_ctx = tc.tile_pool(name="xt", bufs=2)
            xtp = xt_ctx.__enter__()
            xt_tiles = {0: xt0}   # jb0 was prefetched during phase Q

            def fetch_xt(jb):
                t_ = xtp.tile([128, DT, 512], BF, tag="xt")
                nc.sync.dma_start(out=t_, in_=xt[:, jb])
                xt_tiles[jb] = t_

            fetch_xt(1)
            if True:
                pa = None

                def kv_advance(nxt):
                    nonlocal pa
                    if pa is not None:
                        if pa[0] == 'k':
                            _, ps, jsl, dst = pa
                            pq, ss = rms_stage_a(tmpp, ps, ssp)
                            rms_stage_b(tmpp, pq, ss, ck, jsl, dst, None)
                        else:
                            _, dst, psv = pa
                            nc.scalar.copy(dst, psv)
                    pa = nxt

                for jb in range(4):
                    jsl = slice(jb * 512, (jb + 1) * 512)
                    xtt = xt_tiles.pop(jb)
                    if jb + 2 < 4:
                        fetch_xt(jb + 2)
                    for g in range(HKV):
                        ps = psp.tile([128, 512], F32, tag="ps")
                        for dt_ in range(DT):
                            nc.tensor.matmul(
                                ps,
                                lhsT=wkt[:, dt_, g * HD:(g + 1) * HD],
                                rhs=xtt[:, dt_, :],
                                start=(dt_ == 0), stop=(dt_ == DT - 1))
                        kv_advance(('k', ps, jsl, kg_all[:, g, jsl]))
                    for jt in range(4):
                        psv = psp.tile([128, 512], F32, tag="ps")
                        for dt_ in range(DT):
                            nc.tensor.matmul(
                                psv,
                                lhsT=xtt[:, dt_, jt * 128:(jt + 1) * 128],
                                rhs=wvt[:, dt_, :],
                                start=(dt_ == 0), stop=(dt_ == DT - 1))
                        kv_advance(('v', v_all[:, jb * 4 + jt, :], psv))
                kv_advance(None)
            xt_ctx.__exit__(None, None, None)
            ck_ctx.__exit__(None, None, None)
            xt0_ctx.__exit__(None, None, None)
            ss_ctx.__exit__(None, None, None)
            ps_ctx.__exit__(None, None, None)
            rms_ctx.__exit__(None, None, None)
            kvw_ctx.__exit__(None, None, None)

            # ---- Attention + proj (y and proj weights live here) ----
            with tc.tile_pool(name="res2", bufs=1) as res2p:
                # Prefetch all proj weights during attention (resident).
                wpt = res2p.tile([128, 4, DT, 512], BF)
                for og in range(4):
                    nc.sync.dma_start(out=wpt[:, og], in_=wp[:, og])
                y_all = res2p.tile([128, H, R], BF)   # [hd, h, row]

                # ---------------- Phase attention ----------------
                # One step = (head h, q-tile slot i). Steps are software-
                # pipelined: scores+exp+mask of step u are emitted before
                # the ys/normalize of step u-1, so the PE streams while
                # ACT exps the previous step's strips. The causal mask is
                # applied in place on GpSimd; the softmax denominator is
                # tree-folded per strip on DVE (bf16) so the PE only pays
                # one 128-row 1^T-matmul per strip instead of per key
                # tile.
                with tc.tile_pool(name="pts", bufs=4) as ptp, \
                     tc.tile_pool(name="fold", bufs=3) as ftp, \
                     tc.tile_pool(name="ntp", bufs=3) as ntp, \
                     tc.tile_pool(name="sc", bufs=2, space="PSUM") as scp, \
                     tc.tile_pool(name="ys", bufs=3, space="PSUM") as ysp:

                    def emit_scores(g, h, i):
                        """Scores + exp + in-place mask + one fold level
                        (w -> w/2 on DVE, bf16) for step (h, i)."""
                        e = EPROC[i]
                        tsl = slice(i * 128, (i + 1) * 128)
                        pts = []           # (pt_tile, fold_tile, w, kt_base)
                        kt_base = 0
                        for w in _strips(e):
                            sp = scp.tile([128, 8, 128], F32, tag="sc")
                            for k in range(w):
                                kt = kt_base + k
                                nc.tensor.matmul(
                                    sp[:, k, :],
                                    lhsT=kg_all[:, g, kt * 128:(kt + 1) * 128],
                                    rhs=q_all[:, h, tsl],
                                    start=True, stop=True)
                            pt = ptp.tile([128, 8, 128], BF, tag="pt")
                            nc.scalar.activation(pt[:, 0:w, :], sp[:, 0:w, :],
                                                 AF.Exp)
                            if kt_base + w == e:   # strip has the last 2 kts
                                tl = (e - 2) - kt_base
                                nc.vector.tensor_mul(pt[:, tl:tl + 2, :],
                                                     pt[:, tl:tl + 2, :], mqs)
                            fh = ftp.tile([128, 4, 128], BF, tag="fh")
                            nc.vector.tensor_add(fh[:, 0:w // 2, :],
                                                 pt[:, 0:w // 2, :],
                                                 pt[:, w // 2:w, :])
                            pts.append((pt, fh, w, kt_base))
                            kt_base += w
                        return (g, h, i, e, pts)

                    def emit_consume(st):
                        g, h, i, e, pts = st
                        tsl = slice(i * 128, (i + 1) * 128)
                        ys = ysp.tile([128, 512], F32, tag="ys")
                        for pt, fh, w, kt_base in pts:
                            for k in range(w):
                                kt = kt_base + k
                                nc.tensor.matmul(
                                    ys[:, 0:128],
                                    lhsT=v_all[:, kt, g * HD:(g + 1) * HD],
                                    rhs=pt[:, k, :],
                                    start=(kt == 0), stop=(kt == e - 1))
                        nd = e // 2
                        di = 0
                        for pt, fh, w, kt_base in pts:
                            for j in range(w // 2):
                                nc.tensor.matmul(
                                    ys[0:1, 256:384], lhsT=ones,
                                    rhs=fh[:, j, :],
                                    start=(di == 0), stop=(di == nd - 1))
                                di += 1
                        rc = ntp.tile([1, 128], F32, tag="rc")
                        nc.vector.reciprocal_approx_fast(rc, ys[0:1, 256:384])
                        rcb = ntp.tile([128, 128], F32, tag="rcb")
                        nc.gpsimd.partition_broadcast(rcb, rc)
                        nc.vector.tensor_mul(y_all[:, h, tsl], ys[:, 0:128],
                                             rcb)

                    # Slots run in ascending causal extent (2,4,...,16): a
                    # step's scores+exp burst is then always paired with a
                    # same-or-bigger consume of the previous step, so the
                    # PE never outruns ACT at head boundaries, and the
                    # first steps only need the earliest K blocks.
                    prev = None
                    for g in range(HKV):
                        for hh in range(REP):
                            h = g * REP + hh
                            for i in reversed(range(8)):
                                st = emit_scores(g, h, i)
                                if prev is not None:
                                    emit_consume(prev)
                                prev = st
                    emit_consume(prev)

                # ---------------- Phase proj ----------------
                with tc.tile_pool(name="obp", bufs=2) as obp, \
                     tc.tile_pool(name="pso", bufs=2, space="PSUM") as psp:
                    for og in range(4):
                        for oo in range(4):
                            ot = og * 4 + oo
                            for ib in range(2):
                                isl = slice(ib * 512, (ib + 1) * 512)
                                ps = psp.tile([128, 512], F32, tag="o_ps")
                                for ct in range(DT):
                                    nc.tensor.matmul(
                                        ps,
                                        lhsT=wpt[:, og, ct,
                                                 oo * 128:(oo + 1) * 128],
                                        rhs=y_all[:, ct, isl],
                                        start=(ct == 0), stop=(ct == DT - 1))
                                ob = obp.tile([128, 512], F32, tag="ob")
                                nc.vector.tensor_copy(ob, ps)
                                nc.sync.dma_start(
                                    out=outT[ot * 128:(ot + 1) * 128, isl],
                                    in_=ob)

    nc.compile()
    return nc


def _rope_tables():
    inv = (1.0 / (np.float32(ROPE_BASE)
                  ** (np.arange(0, HD, 2, dtype=np.float32) / np.float32(HD))))
    t = np.arange(T, dtype=np.float32)
    freqs = np.outer(t, inv).astype(np.float32)          # [T, 64]
    c, si = np.cos(freqs).T, np.sin(freqs).T             # [64, T]
    # rows 0..63 twice for cos; +sin rows then -sin rows: with qn-halves
    # swapped this computes (q1*c + q2*s, q2*c - q1*s) in aligned DVE ops.
    cos_full = np.ascontiguousarray(np.concatenate([c, c], axis=0))
    sin_signed = np.ascontiguousarray(np.concatenate([si, -si], axis=0))
    return cos_full, sin_signed


def _proc_tiles(half):
    """q-tile (128-row block) indices in processing order: extent of slot
    i must be <= EPROC[i]."""
    return [e - 2 for e in EPROC] if half == 0 else [e - 1 for e in EPROC]


def _mask(half):
    """[128 key, 2, 128 row] {0,1} bf16 mask for the last 2 key tiles of
    every strip: half0 -> [tri, 0], half1 -> [1, tri]."""
    jj = np.arange(128)[:, None]
    rr = np.arange(128)[None, :]
    tri = (jj <= rr).astype(BF16)
    m = np.zeros((128, 2, 128), dtype=BF16)
    if half == 0:
        m[:, 0] = tri
    else:
        m[:, 0] = 1
        m[:, 1] = tri
    return np.ascontiguousarray(m.reshape(128, 256))


def _pdt(aT):
    """[dt*128, N] -> [128, dt, N] (partition-major, contiguous per part)."""
    d, n = aT.shape
    return np.ascontiguousarray(aT.reshape(d // 128, 128, n).transpose(1, 0, 2))


def kernel(**inputs):
    from concourse.bass_utils import run_bass_kernel_spmd

    x = np.ascontiguousarray(np.asarray(inputs["x"], dtype=np.float32))
    Wq = np.asarray(inputs["Wq"], dtype=np.float32)
    Wk = np.asarray(inputs["Wk"], dtype=np.float32)
    Wv = np.asarray(inputs["Wv"], dtype=np.float32)
    Wproj = np.asarray(inputs["Wproj"], dtype=np.float32)
    q_gain = np.ascontiguousarray(np.asarray(inputs["q_gain"], dtype=np.float32))

    if "nc" not in _CACHE:
        _CACHE["nc"] = _build()
    nc = _CACHE["nc"]

    def tb(a):  # transpose + bf16, contiguous
        return np.ascontiguousarray(a.T.astype(BF16))

    # wq: [128, dt, 2048] -> [128, hg=8, dt, 256] group-major
    wq_a = _pdt(tb(Wq)).reshape(128, DT, 8, 256).transpose(0, 2, 1, 3)
    wq_a = np.ascontiguousarray(wq_a)
    wk_a = _pdt(tb(Wk))
    wv_a = _pdt(tb(Wv))
    # wp: [128, ct, 2048] -> [128, og=4, ct, 512]
    wp_a = _pdt(tb(Wproj)).reshape(128, DT, 4, 512).transpose(0, 2, 1, 3)
    wp_a = np.ascontiguousarray(wp_a)
    cosT, sinT = _rope_tables()

    in_maps = []
    for c in range(8):
        b, half = divmod(c, 2)
        tiles = _proc_tiles(half)
        ridx = np.concatenate([np.arange(t * 128, (t + 1) * 128) for t in tiles])
        xb = x[b]
        # xt: [128, dt, 2048 tokens] -> [128, jb=4, dt, 512]
        xt_a = _pdt(tb(xb)).reshape(128, DT, 4, 512).transpose(0, 2, 1, 3)
        in_maps.append({
            "xq": _pdt(tb(xb[ridx])),
            "xt": np.ascontiguousarray(xt_a),
            "wq": wq_a, "wk": wk_a, "wv": wv_a, "wp": wp_a,
            "qgain": q_gain,
            "cosq": np.ascontiguousarray(cosT[:, ridx]),
            "sinq": np.ascontiguousarray(sinT[:, ridx]),
            "cosk": cosT, "sink": sinT,
            "mq": _mask(half),
        })

    res = run_bass_kernel_spmd(nc, in_maps, core_ids=list(range(8)),
                               tmpdir=os.environ.get("BASS_KERNEL_TMPDIR"))
    _CACHE["res"] = res

    out = np.empty((B, T, DIM), dtype=np.float32)
    for c in range(8):
        b, half = divmod(c, 2)
        oT = res.results[c]["outT"]
        for i, t in enumerate(_proc_tiles(half)):
            out[b, t * 128:(t + 1) * 128] = oT[:, i * 128:(i + 1) * 128].T
    return out


# revision 44
# speedup vs baseline: 1.0477x; 1.0065x over previous
"""Causal self-attention (GQA + RMSNorm + RoPE) Trainium2 Bass kernel.

Sharding: data-parallel over (batch, q-rows). 8 cores = 4 batches x 2 row
sets. Each core computes full K/V for its batch and 1024 q rows chosen as
8 x 128-row tiles: core half 0 takes even tiles, half 1 odd tiles. Tiles
are processed in descending causal-extent order so that a single
compile-time key-extent schedule E = (16,14,12,10,8,6,4,2) (in 128-key
tiles) is an upper bound for both halves: total scored coverage is 72
units/head vs 68 ideal causal, vs 96 dense-halves. No collectives.

All DRAM operands are pre-arranged on the host into [128-partition,
chunk, free] layouts so every load is one contiguous run per partition
(128 descriptors instead of thousands). K-phase weights prefetch during
phase Q; proj weights prefetch during attention.

On-chip layout is channel-major: scores are computed key-major
(S^T tile = K_tile^T.T @ Q^T) in [128,8,128] PSUM strips, exp'd in one
wide ACT instruction per strip, causal-masked in place by a {0,1}
multiply on only the last two key tiles (diagonal triangle + optional
padding), and consumed by per-q-tile accumulating ys (V^T @ P) matmul
chains. The softmax denominator does one DVE fold level per strip
(w -> w/2 pairwise adds, bf16 2x mode) so the PE only streams e/2
1^T-matmuls per step instead of e. Normalization (reciprocal_approx +
gpsimd partition broadcast) is applied to the y tile.

The QKV rms/rope epilogue copies each PSUM tile to SBUF with one ACT op
immediately after the matmul chain stops, so the PE's PSUM buffer is
released after two quick ACT reads instead of being held through the
whole serial sqrt/recip/broadcast/rope chain (which previously stalled
the PE ~4us per head). The whole attention pipeline is software-
pipelined one (head, q-tile) step deep. Q stays SBUF-resident between
phases. All matmul operands bf16 (fp32 accumulate); softmax/statistics
math fp32.
"""

import os
import sys

sys.path.insert(0, "/opt/trn_rl_repo")

import ml_dtypes
import numpy as np

DIM = 2048
H = 16
HKV = 4
HD = 128
REP = H // HKV
B = 4
T = 2048
R = 1024          # q rows per core
DT = DIM // 128   # 16 contraction tiles
NJT = T // 128    # 16 key tiles
KVD = HKV * HD    # 512
EPROC = (16, 14, 12, 10, 8, 6, 4, 2)  # key-tile extent per q-tile slot
ROPE_BASE = 10000.0
EPS = float(np.finfo(np.float32).eps)
BF16 = ml_dtypes.bfloat16

_CACHE = {}


def _strips(e):
    """Split an extent into PSUM-strip chunk widths (max 8 key tiles)."""
    out = [8] * (e // 8)
    if e % 8:
        out.append(e % 8)
    return out


def _build():
    """Build + compile the SPMD Bass program (once per process)."""
    from concourse import bacc
    import concourse.mybir as mybir
    import concourse.tile as tile

    F32 = mybir.dt.float32
    BF = mybir.dt.bfloat16
    AF = mybir.ActivationFunctionType

    nc = bacc.Bacc("TRN2", target_bir_lowering=False, debug=False)

    # All tensors pre-arranged host-side: partition dim first, contiguous
    # free bytes per partition for every dma slice taken below.
    xq = nc.dram_tensor("xq", [128, DT, R], BF, kind="ExternalInput")
    xt = nc.dram_tensor("xt", [128, 4, DT, 512], BF, kind="ExternalInput")
    wq = nc.dram_tensor("wq", [128, 8, DT, 256], BF, kind="ExternalInput")
    wk = nc.dram_tensor("wk", [128, DT, KVD], BF, kind="ExternalInput")
    wv = nc.dram_tensor("wv", [128, DT, KVD], BF, kind="ExternalInput")
    wp = nc.dram_tensor("wp", [128, 4, DT, 512], BF, kind="ExternalInput")
    qgain = nc.dram_tensor("qgain", [H], F32, kind="ExternalInput")
    cosq = nc.dram_tensor("cosq", [HD, R], F32, kind="ExternalInput")
    sinq = nc.dram_tensor("sinq", [HD, R], F32, kind="ExternalInput")
    cosk = nc.dram_tensor("cosk", [HD, T], F32, kind="ExternalInput")
    sink = nc.dram_tensor("sink", [HD, T], F32, kind="ExternalInput")
    # per-core {0,1} mask for the last two key tiles of every q-tile strip:
    # half0 -> [tri, 0], half1 -> [1, tri]  (key-major [key, 2, row])
    mq = nc.dram_tensor("mq", [128, 2 * 128], BF, kind="ExternalInput")
    outT = nc.dram_tensor("outT", [DIM, R], F32, kind="ExternalOutput")

    with tile.TileContext(nc) as tc:
        with tc.tile_pool(name="const", bufs=1) as constp, \
             tc.tile_pool(name="res", bufs=1) as resp:
            # Pool stack (LIFO release): kvw, ck, rms span Q+KV; xq/wq/cq
            # are Q-only and sit on top so they can be released after Q.
            kvw_ctx = tc.tile_pool(name="kvw", bufs=1)
            kvwp = kvw_ctx.__enter__()
            rms_ctx = tc.tile_pool(name="rms", bufs=2)
            tmpp = rms_ctx.__enter__()
            ps_ctx = tc.tile_pool(name="psqkv", bufs=4, space="PSUM")
            psp = ps_ctx.__enter__()
            ss_ctx = tc.tile_pool(name="ssqkv", bufs=4, space="PSUM")
            ssp = ss_ctx.__enter__()
            xt0_ctx = tc.tile_pool(name="xt0", bufs=1)
            xt0p = xt0_ctx.__enter__()
            xq_ctx = tc.tile_pool(name="xq", bufs=1)
            xqp = xq_ctx.__enter__()
            wq_ctx = tc.tile_pool(name="wq", bufs=2)
            wqp = wq_ctx.__enter__()
            cq_ctx = tc.tile_pool(name="cq", bufs=1)
            cqp = cq_ctx.__enter__()

            # ---- Phase-Q critical loads first (in Sync program order) ----
            # Leading chunks are small so the first matmul's deps land fast.
            xqt = xqp.tile([128, DT, R], BF)
            wq0 = wqp.tile([128, DT, 256], BF, tag="wq")
            for lo, hi in ((0, 2), (2, 4)):
                nc.sync.dma_start(out=xqt[:, lo:hi, :], in_=xq[:, lo:hi, :])
                nc.sync.dma_start(out=wq0[:, lo:hi, :], in_=wq[:, 0, lo:hi, :])
            nc.sync.dma_start(out=wq0[:, 4:DT, :], in_=wq[:, 0, 4:DT, :])
            cq = cqp.tile([128, 2, R], F32)
            nc.sync.dma_start(out=cq[:, 0], in_=cosq[:, :])
            nc.sync.dma_start(out=cq[:, 1], in_=sinq[:, :])
            for ch in range(1, 4):
                nc.sync.dma_start(out=xqt[:, ch * 4:(ch + 1) * 4, :],
                                  in_=xq[:, ch * 4:(ch + 1) * 4, :])

            # K/V-phase weights + first-token-block prefetch: tiles
            # allocated here, dma_starts interleaved into the phase-Q loop
            # so queue order stays behind the urgent per-group wq loads.
            wkt = kvwp.tile([128, DT, KVD], BF)
            wvt = kvwp.tile([128, DT, KVD], BF)
            xt0 = xt0p.tile([128, DT, 512], BF)

            def prefetch_kv(hg):
                if hg == 4:
                    nc.sync.dma_start(out=wkt, in_=wk[:, :, :])
                elif hg == 5:
                    nc.sync.dma_start(out=wvt, in_=wv[:, :, :])
                elif hg == 6:
                    nc.sync.dma_start(out=xt0, in_=xt[:, 0])

            # ---- Constants ----
            ones = constp.tile([128, 1], BF)
            nc.vector.memset(ones, 1.0)
            qg = constp.tile([1, H], F32)
            nc.sync.dma_start(out=qg, in_=qgain[None, :])
            epsq = constp.tile([1, 1], F32)
            nc.vector.memset(epsq, EPS * HD)   # q scale: 1/sqrt(ssum + HD*eps)
            epsk = constp.tile([1, 1], F32)
            nc.vector.memset(epsk, EPS)        # k scale: rsqrt(ssum/HD + eps)
            mqs = constp.tile([128, 2, 128], BF)
            nc.sync.dma_start(out=mqs, in_=mq.rearrange("p (t r) -> p t r", t=2))

            # SBUF residents across phases
            q_all = resp.tile([128, H, R], BF)        # [hd, h, row]
            kg_all = resp.tile([128, HKV, T], BF)     # [hd, g, key]
            v_all = resp.tile([128, NJT, KVD], BF)    # [key%128, kt, c]

            # rms-normalize PSUM tile [128,512] per token, rope, write bf16
            # to dst AP. For q, 1/sqrt(HD) and head gain fold into the scale.
            # Two pipeline stages: A (square + copy + sum-of-squares matmul)
            # releases the PSUM tile right away; B (sqrt/recip/broadcast/
            # rope) runs one step later so ACT's sqrt never sits in front
            # of the next tile's square in the ACT queue.
            def rms_stage_a(tmpp, ps, ssp):
                sq = tmpp.tile([128, 512], BF, tag="rr_sq")
                nc.scalar.square(sq, ps)
                pq = tmpp.tile([128, 512], F32, tag="rr_pq")
                nc.scalar.copy(pq, ps)
                ss = ssp.tile([1, 512], F32, tag="rr_ss")
                nc.tensor.matmul(ss, lhsT=ones, rhs=sq, start=True, stop=True)
                return pq, ss

            def rms_stage_b(tmpp, pq, ss, cs, isl, dst, gain_ap):
                sq_s = tmpp.tile([1, 512], F32, tag="rr_sqs")
                scl = tmpp.tile([1, 512], F32, tag="rr_scl")
                if gain_ap is not None:
                    nc.scalar.activation(sq_s, ss, AF.Sqrt, bias=epsq[0:1, 0:1])
                    nc.vector.reciprocal_approx_fast(scl, sq_s)
                    nc.vector.tensor_scalar_mul(scl, in0=scl, scalar1=gain_ap)
                else:
                    nc.scalar.activation(sq_s, ss, AF.Sqrt, bias=epsk[0:1, 0:1],
                                         scale=1.0 / HD)
                    nc.vector.reciprocal_approx_fast(scl, sq_s)
                sclb = tmpp.tile([128, 512], F32, tag="rr_sclb")
                nc.gpsimd.partition_broadcast(sclb, scl)
                qn = tmpp.tile([128, 512], F32, tag="rr_qn")
                nc.vector.tensor_mul(qn, pq, sclb)
                qnsw = tmpp.tile([128, 512], F32, tag="rr_qnsw")
                nc.sync.dma_start(out=qnsw[0:64], in_=qn[64:128])
                nc.sync.dma_start(out=qnsw[64:128], in_=qn[0:64])
                t12 = tmpp.tile([128, 512], BF, tag="rr_t12")
                nc.vector.tensor_mul(t12, qn, cs[:, 0, isl])
                t34 = tmpp.tile([128, 512], BF, tag="rr_t34")
                nc.vector.tensor_mul(t34, qnsw, cs[:, 1, isl])
                nc.vector.tensor_add(dst, t12, t34)

            # ---------------- Phase Q ----------------
            if True:
                pa = pb = None
                for hg in range(8):   # 2 heads per weight group
                    if hg == 0:
                        wqt = wq0
                    else:
                        wqt = wqp.tile([128, DT, 256], BF, tag="wq")
                        nc.sync.dma_start(out=wqt, in_=wq[:, hg])
                    prefetch_kv(hg)
                    for hh in range(2):
                        h = hg * 2 + hh
                        for ib in range(2):
                            isl = slice(ib * 512, (ib + 1) * 512)
                            ps = psp.tile([128, 512], F32, tag="ps")
                            for dt_ in range(DT):
                                nc.tensor.matmul(
                                    ps,
                                    lhsT=wqt[:, dt_, hh * HD:(hh + 1) * HD],
                                    rhs=xqt[:, dt_, isl],
                                    start=(dt_ == 0), stop=(dt_ == DT - 1))
                            if pa is not None:
                                pq, ss = rms_stage_a(tmpp, pa[0], ssp)
                                rms_stage_b(tmpp, pq, ss, *pa[1:])
                            pa = (ps, cq, isl,
                                  q_all[:, h, isl], qg[0:1, h:h + 1])
                pq, ss = rms_stage_a(tmpp, pa[0], ssp)
                rms_stage_b(tmpp, pq, ss, *pa[1:])
            cq_ctx.__exit__(None, None, None)
            wq_ctx.__exit__(None, None, None)
            xq_ctx.__exit__(None, None, None)

            # ---------------- Phase K/V ----------------
            ck_ctx = tc.tile_pool(name="ckp", bufs=1)
            ckp = ck_ctx.__enter__()
            ck = ckp.tile([128, 2, T], F32)
            nc.sync.dma_start(out=ck[:, 0], in_=cosk[:, :])
            nc.sync.dma_start(out=ck[:, 1], in_=sink[:, :])

            xt_ctx = tc.tile_pool(name="xt", bufs=2)
            xtp = xt_ctx.__enter__()
            xt_tiles = {0: xt0}   # jb0 was prefetched during phase Q

            def fetch_xt(jb):
                t_ = xtp.tile([128, DT, 512], BF, tag="xt")
                nc.sync.dma_start(out=t_, in_=xt[:, jb])
                xt_tiles[jb] = t_

            fetch_xt(1)
            if True:
                pa = None

                def kv_advance(nxt):
                    nonlocal pa
                    if pa is not None:
                        if pa[0] == 'k':
                            _, ps, jsl, dst = pa
                            pq, ss = rms_stage_a(tmpp, ps, ssp)
                            rms_stage_b(tmpp, pq, ss, ck, jsl, dst, None)
                        else:
                            _, dst, psv = pa
                            nc.scalar.copy(dst, psv)
                    pa = nxt

                for jb in range(4):
                    jsl = slice(jb * 512, (jb + 1) * 512)
                    xtt = xt_tiles.pop(jb)
                    if jb + 2 < 4:
                        fetch_xt(jb + 2)
                    for g in range(HKV):
                        ps = psp.tile([128, 512], F32, tag="ps")
                        for dt_ in range(DT):
                            nc.tensor.matmul(
                                ps,
                                lhsT=wkt[:, dt_, g * HD:(g + 1) * HD],
                                rhs=xtt[:, dt_, :],
                                start=(dt_ == 0), stop=(dt_ == DT - 1))
                        kv_advance(('k', ps, jsl, kg_all[:, g, jsl]))
                    for jt in range(4):
                        psv = psp.tile([128, 512], F32, tag="ps")
                        for dt_ in range(DT):
                            nc.tensor.matmul(
                                psv,
                                lhsT=xtt[:, dt_, jt * 128:(jt + 1) * 128],
                                rhs=wvt[:, dt_, :],
                                start=(dt_ == 0), stop=(dt_ == DT - 1))
                        kv_advance(('v', v_all[:, jb * 4 + jt, :], psv))
                kv_advance(None)
            xt_ctx.__exit__(None, None, None)
            ck_ctx.__exit__(None, None, None)
            xt0_ctx.__exit__(None, None, None)
            ss_ctx.__exit__(None, None, None)
            ps_ctx.__exit__(None, None, None)
            rms_ctx.__exit__(None, None, None)
            kvw_ctx.__exit__(None, None, None)

            # ---- Attention + proj (y and proj weights live here) ----
            with tc.tile_pool(name="res2", bufs=1) as res2p:
                # Prefetch all proj weights during attention (resident).
                wpt = res2p.tile([128, 4, DT, 512], BF)
                for og in range(4):
                    nc.sync.dma_start(out=wpt[:, og], in_=wp[:, og])
                y_all = res2p.tile([128, H, R], BF)   # [hd, h, row]

                # ---------------- Phase attention ----------------
                # One step = (head h, q-tile slot i). Steps are software-
                # pipelined: scores+exp+mask of step u are emitted before
                # the ys/normalize of step u-1, so the PE streams while
                # ACT exps the previous step's strips. The causal mask is
                # applied in place on GpSimd; the softmax denominator is
                # tree-folded per strip on DVE (bf16) so the PE only pays
                # one 128-row 1^T-matmul per strip instead of per key
                # tile.
                with tc.tile_pool(name="pts", bufs=4) as ptp, \
                     tc.tile_pool(name="fold", bufs=3) as ftp, \
                     tc.tile_pool(name="ntp", bufs=3) as ntp, \
                     tc.tile_pool(name="sc", bufs=2, space="PSUM") as scp, \
                     tc.tile_pool(name="ys", bufs=3, space="PSUM") as ysp:

                    def emit_scores(g, h, i):
                        """Scores + exp + in-place mask + one fold level
                        (w -> w/2 on DVE, bf16) for step (h, i)."""
                        e = EPROC[i]
                        tsl = slice(i * 128, (i + 1) * 128)
                        pts = []           # (pt_tile, fold_tile, w, kt_base)
                        kt_base = 0
                        for w in _strips(e):
                            sp = scp.tile([128, 8, 128], F32, tag="sc")
                            for k in range(w):
                                kt = kt_base + k
                                nc.tensor.matmul(
                                    sp[:, k, :],
                                    lhsT=kg_all[:, g, kt * 128:(kt + 1) * 128],
                                    rhs=q_all[:, h, tsl],
                                    start=True, stop=True)
                            pt = ptp.tile([128, 8, 128], BF, tag="pt")
                            nc.scalar.activation(pt[:, 0:w, :], sp[:, 0:w, :],
                                                 AF.Exp)
                            if kt_base + w == e:   # strip has the last 2 kts
                                tl = (e - 2) - kt_base
                                nc.vector.tensor_mul(pt[:, tl:tl + 2, :],
                                                     pt[:, tl:tl + 2, :], mqs)
                            fh = ftp.tile([128, 4, 128], BF, tag="fh")
                            nc.vector.tensor_add(fh[:, 0:w // 2, :],
                                                 pt[:, 0:w // 2, :],
                                                 pt[:, w // 2:w, :])
                            pts.append((pt, fh, w, kt_base))
                            kt_base += w
                        return (g, h, i, e, pts)

                    def emit_consume(st):
                        g, h, i, e, pts = st
                        tsl = slice(i * 128, (i + 1) * 128)
                        ys = ysp.tile([128, 512], F32, tag="ys")
                        for pt, fh, w, kt_base in pts:
                            for k in range(w):
                                kt = kt_base + k
                                nc.tensor.matmul(
                                    ys[:, 0:128],
                                    lhsT=v_all[:, kt, g * HD:(g + 1) * HD],
                                    rhs=pt[:, k, :],
                                    start=(kt == 0), stop=(kt == e - 1))
                        # second fold level (DVE): <=2 summand tiles per
                        # strip, so the PE streams at most 2 denominator
                        # matmuls per strip instead of w/2.
                        nd_aps = []
                        for pt, fh, w, kt_base in pts:
                            w2 = w // 2
                            if w2 == 1:
                                nd_aps.append(fh[:, 0, :])
                                continue
                            fh2 = ftp.tile([128, 2, 128], BF, tag="fh2")
                            if w2 == 4:
                                nc.vector.tensor_add(fh2, fh[:, 0:2, :],
                                                     fh[:, 2:4, :])
                                nd_aps += [fh2[:, 0, :], fh2[:, 1, :]]
                            elif w2 == 3:
                                nc.vector.tensor_add(fh2[:, 0, :],
                                                     fh[:, 0, :], fh[:, 1, :])
                                nd_aps += [fh2[:, 0, :], fh[:, 2, :]]
                            else:  # w2 == 2
                                nc.vector.tensor_add(fh2[:, 0, :],
                                                     fh[:, 0, :], fh[:, 1, :])
                                nd_aps.append(fh2[:, 0, :])
                        for di, ap in enumerate(nd_aps):
                            nc.tensor.matmul(
                                ys[0:1, 256:384], lhsT=ones, rhs=ap,
                                start=(di == 0),
                                stop=(di == len(nd_aps) - 1))
                        rc = ntp.tile([1, 128], F32, tag="rc")
                        nc.vector.reciprocal_approx_fast(rc, ys[0:1, 256:384])
                        rcb = ntp.tile([128, 128], F32, tag="rcb")
                        nc.gpsimd.partition_broadcast(rcb, rc)
                        nc.vector.tensor_mul(y_all[:, h, tsl], ys[:, 0:128],
                                             rcb)

                    # Slots run in ascending causal extent (2,4,...,16): a
                    # step's scores+exp burst is then always paired with a
                    # same-or-bigger consume of the previous step, so the
                    # PE never outruns ACT at head boundaries, and the
                    # first steps only need the earliest K blocks.
                    prev = None
                    for g in range(HKV):
                        for hh in range(REP):
                            h = g * REP + hh
                            for i in reversed(range(8)):
                                st = emit_scores(g, h, i)
                                if prev is not None:
                                    emit_consume(prev)
                                prev = st
                    emit_consume(prev)

                # ---------------- Phase proj ----------------
                with tc.tile_pool(name="obp", bufs=2) as obp, \
                     tc.tile_pool(name="pso", bufs=2, space="PSUM") as psp:
                    for og in range(4):
                        for oo in range(4):
                            ot = og * 4 + oo
                            for ib in range(2):
                                isl = slice(ib * 512, (ib + 1) * 512)
                                ps = psp.tile([128, 512], F32, tag="o_ps")
                                for ct in range(DT):
                                    nc.tensor.matmul(
                                        ps,
                                        lhsT=wpt[:, og, ct,
                                                 oo * 128:(oo + 1) * 128],
                                        rhs=y_all[:, ct, isl],
                                        start=(ct == 0), stop=(ct == DT - 1))
                                ob = obp.tile([128, 512], F32, tag="ob")
                                nc.vector.tensor_copy(ob, ps)
                                nc.sync.dma_start(
                                    out=outT[ot * 128:(ot + 1) * 128, isl],
                                    in_=ob)

    nc.compile()
    return nc


def _rope_tables():
    inv = (1.0 / (np.float32(ROPE_BASE)
                  ** (np.arange(0, HD, 2, dtype=np.float32) / np.float32(HD))))
    t = np.arange(T, dtype=np.float32)
    freqs = np.outer(t, inv).astype(np.float32)          # [T, 64]
    c, si = np.cos(freqs).T, np.sin(freqs).T             # [64, T]
    # rows 0..63 twice for cos; +sin rows then -sin rows: with qn-halves
    # swapped this computes (q1*c + q2*s, q2*c - q1*s) in aligned DVE ops.
    cos_full = np.ascontiguousarray(np.concatenate([c, c], axis=0))
    sin_signed = np.ascontiguousarray(np.concatenate([si, -si], axis=0))
    return cos_full, sin_signed


def _proc_tiles(half):
    """q-tile (128-row block) indices in processing order: extent of slot
    i must be <= EPROC[i]."""
    return [e - 2 for e in EPROC] if half == 0 else [e - 1 for e in EPROC]


def _mask(half):
    """[128 key, 2, 128 row] {0,1} bf16 mask for the last 2 key tiles of
    every strip: half0 -> [tri, 0], half1 -> [1, tri]."""
    jj = np.arange(128)[:, None]
    rr = np.arange(128)[None, :]
    tri = (jj <= rr).astype(BF16)
    m = np.zeros((128, 2, 128), dtype=BF16)
    if half == 0:
        m[:, 0] = tri
    else:
        m[:, 0] = 1
        m[:, 1] = tri
    return np.ascontiguousarray(m.reshape(128, 256))


def _pdt(aT):
    """[dt*128, N] -> [128, dt, N] (partition-major, contiguous per part)."""
    d, n = aT.shape
    return np.ascontiguousarray(aT.reshape(d // 128, 128, n).transpose(1, 0, 2))


def kernel(**inputs):
    from concourse.bass_utils import run_bass_kernel_spmd

    x = np.ascontiguousarray(np.asarray(inputs["x"], dtype=np.float32))
    Wq = np.asarray(inputs["Wq"], dtype=np.float32)
    Wk = np.asarray(inputs["Wk"], dtype=np.float32)
    Wv = np.asarray(inputs["Wv"], dtype=np.float32)
    Wproj = np.asarray(inputs["Wproj"], dtype=np.float32)
    q_gain = np.ascontiguousarray(np.asarray(inputs["q_gain"], dtype=np.float32))

    if "nc" not in _CACHE:
        _CACHE["nc"] = _build()
    nc = _CACHE["nc"]

    def tb(a):  # transpose + bf16, contiguous
        return np.ascontiguousarray(a.T.astype(BF16))

    # wq: [128, dt, 2048] -> [128, hg=8, dt, 256] group-major
    wq_a = _pdt(tb(Wq)).reshape(128, DT, 8, 256).transpose(0, 2, 1, 3)
    wq_a = np.ascontiguousarray(wq_a)
    wk_a = _pdt(tb(Wk))
    wv_a = _pdt(tb(Wv))
    # wp: [128, ct, 2048] -> [128, og=4, ct, 512]
    wp_a = _pdt(tb(Wproj)).reshape(128, DT, 4, 512).transpose(0, 2, 1, 3)
    wp_a = np.ascontiguousarray(wp_a)
    cosT, sinT = _rope_tables()

    in_maps = []
    for c in range(8):
        b, half = divmod(c, 2)
        tiles = _proc_tiles(half)
        ridx = np.concatenate([np.arange(t * 128, (t + 1) * 128) for t in tiles])
        xb = x[b]
        # xt: [128, dt, 2048 tokens] -> [128, jb=4, dt, 512]
        xt_a = _pdt(tb(xb)).reshape(128, DT, 4, 512).transpose(0, 2, 1, 3)
        in_maps.append({
            "xq": _pdt(tb(xb[ridx])),
            "xt": np.ascontiguousarray(xt_a),
            "wq": wq_a, "wk": wk_a, "wv": wv_a, "wp": wp_a,
            "qgain": q_gain,
            "cosq": np.ascontiguousarray(cosT[:, ridx]),
            "sinq": np.ascontiguousarray(sinT[:, ridx]),
            "cosk": cosT, "sink": sinT,
            "mq": _mask(half),
        })

    res = run_bass_kernel_spmd(nc, in_maps, core_ids=list(range(8)),
                               tmpdir=os.environ.get("BASS_KERNEL_TMPDIR"))
    _CACHE["res"] = res

    out = np.empty((B, T, DIM), dtype=np.float32)
    for c in range(8):
        b, half = divmod(c, 2)
        oT = res.results[c]["outT"]
        for i, t in enumerate(_proc_tiles(half)):
            out[b, t * 128:(t + 1) * 128] = oT[:, i * 128:(i + 1) * 128].T
    return out
